# revision 54
# baseline (speedup 1.0000x reference)
"""GATv2 layer kernel for Trainium2 — 8 NeuronCores, SPMD row-sharded.

Math (reference):
    h = x @ W
    s1 = h @ a[:F];  s2 = h @ a[F:]
    e  = leaky_relu(s1[:,None] + s2[None,:], 0.2)
    e  = where(adj > 0, e, -9e15)
    att = softmax(e, axis=1)
    out = elu(att @ h)

Kernel strategy (per core, rows of adj/out sharded across 8 cores; x and
adj columns are rotated per core on the host so each core's own rows are
always chunks 0..SUB-1 — one SPMD program, no separate xs input):
  - s1/s2 are separable: s1 = x @ (W @ a1), s2 = x @ (W @ a2); each core
    computes full h (fp16) from the replicated (rotated) x.
  - exponents are tiny (|s1+s2| <~ 5) so softmax needs no max-subtraction:
    P = adj * exp(lrelu(z)) realized as exp(lrelu(z + adjL)) with
    adjL = (adj-1)*60000 (exp of ~-1.2e4 underflows to exactly 0).
  - the int32->fp16 cast of adj folds BOTH the mask affine and the s1 bias
    in one gpsimd pass: zm = adj*BIG + (s1 - BIG)  (per-partition ptr bias).
  - zm tiles are PE-transposed to [j, i] layout BEFORE the nonlinearity, so
    lrelu(+s2)/exp run in transposed orientation and exp writes the
    attention-matmul rhs (P^T) straight to SBUF — no PSUM->SBUF copy pass.
  - softmax row sums come from a ones-vector matmul accumulated in PSUM
    alongside the attention matmul (both PSUM rows share one bank).
  - j blocks have VARIABLE width ([2,2,4] chunks then 8-chunk blocks): the
    first exp group only needs a 2-chunk column block, so the ACT engine
    starts ~8us earlier than with uniform 8-chunk blocks.
  - all input DMAs are issued from the SP queue in exact consumption order
    (x block 0, adj block 0, ...); emission is deadline-sorted so every
    engine's in-order queue sees pipeline stages in data-arrival order.
  - final: normalize + elu in h'^T orientation as a few big [128, 512] ops
    (rowsum reciprocal is broadcast across partitions with a rank-1 ones
    matmul), then transpose and stream to DRAM.
"""

import sys

if "/opt/trn_rl_repo" not in sys.path:
    sys.path.insert(0, "/opt/trn_rl_repo")

from contextlib import ExitStack

import numpy as np

import concourse.bass as bass
import concourse.tile as tile
from concourse import bacc, mybir
from concourse.masks import make_identity

F32 = mybir.dt.float32
F16 = mybir.dt.float16
I32 = mybir.dt.int32
AF = mybir.ActivationFunctionType
OP = mybir.AluOpType

N_FULL = 8192
F_IN = 256
F_OUT = 128
N_CORES = 8
NEG_SLOPE = 0.2
MASK_BIG = 60000.0  # exactly representable in fp16; exp(-0.2*60000) == 0


def build_gat(
    n=N_FULL,
    rows=N_FULL // N_CORES,
    f_in=F_IN,
    f_out=F_OUT,
    blks=(2, 2, 4, 8, 8, 8, 8, 8, 8, 8),   # chunks per j block
    dve_every=2,     # every k-th chunk routes lrelu to DVE instead of ACT
    cast_split="ppdpppdp",  # cast engine by row-tile index (p=Pool, a=ACT, d=DVE)
    zm_ring={"zmp": 12, "zma": 0, "zmd": 4, "zms": 4},
    p_dt=F16,
    adj_bufs=6,
    zm_bufs=20,
    pt_bufs=6,
    ep_bufs=6,
    tq_bufs=3,
    pa_bufs=2,
    x_bufs=5,
    xt_bufs=4,
    eg=2,
    la_x=12,         # emission lookaheads, in global 128-col chunks
    la_adj=8,
    la_a=26,
    la_cast=6,
    mm_delay=2,      # groups of slack between exp and its attention matmuls
    tq_lead=1,       # extra chunks the tq transposes run ahead of prelu/exp
):
    """Build the per-core Bass program. All cores run the identical program;
    per-core behavior comes only from per-core input data (adj shard + the
    host-side rotation of x / adj columns). Returns the compiled module."""
    KC = f_in // 128          # k chunks of f_in
    NCH = n // 128            # column chunks of adj / row chunks of h
    SUB = rows // 128         # i subtiles per core
    I_BLK = min(512, rows)
    NIH = rows // I_BLK       # i halves for matmul psum banks
    FO2 = f_out + 2           # h columns + [s1 s2]
    EG = eg                   # chunks per exp/matmul-delay group
    XCH = 8                   # chunks per x DMA block
    BLKS = list(blks)
    assert sum(BLKS) == NCH and all(c % EG == 0 for c in BLKS)
    CH0 = [0]
    for c in BLKS:
        CH0.append(CH0[-1] + c)

    nc = bacc.Bacc(
        "TRN2",
        target_bir_lowering=False,
        debug=False,
        enable_asserts=False,
        num_devices=1,
    )
    x_ap = nc.dram_tensor("x", [n, f_in], F32, kind="ExternalInput").ap()
    w_ap = nc.dram_tensor("w", [f_in, f_out], F32, kind="ExternalInput").ap()
    a_ap = nc.dram_tensor("a", [2 * f_out, 1], F32, kind="ExternalInput").ap()
    adj_ap = nc.dram_tensor("adj", [rows, n], I32, kind="ExternalInput").ap()
    out_ap = nc.dram_tensor("out", [rows, f_out], F32, kind="ExternalOutput").ap()

    def dram3(ap, off, dims):
        return bass.AP(tensor=ap.tensor, offset=ap.offset + off, ap=dims)

    with tile.TileContext(nc) as tc, ExitStack() as ctx:
        singles = ctx.enter_context(tc.tile_pool(name="singles", bufs=1))

        rhsW = singles.tile([128, KC * FO2], F32)   # per kc: [W chunk | w1 w2]
        ident32 = singles.tile([128, 128], F32)
        make_identity(nc, ident32)
        identp = singles.tile([128, 128], p_dt)
        make_identity(nc, identp)
        h_sb = singles.tile([128, NCH * f_out], p_dt)
        s2st = singles.tile([128, NCH], F32)     # s2[j] in [j%128, j//128]
        s1m = singles.tile([128, SUB], F32)      # s1 - BIG (cast bias ptr)
        ones128 = singles.tile([128, 128], p_dt)
        scratch = singles.tile([128, f_out], F32)
        a1b = singles.tile([128, f_out], F32)
        a2b = singles.tile([128, f_out], F32)

        # ---- constants: rhsW leads the SP DMA queue (everything chains off
        # rhsW16 -> h -> s1m -> casts); a1b/a2b arrive in parallel on ACT ----
        nc.gpsimd.memset(ones128, 1.0)
        nc.scalar.dma_start(a1b, dram3(a_ap, 0, [[0, 128], [1, f_out]]))
        nc.scalar.dma_start(a2b, dram3(a_ap, f_out, [[0, 128], [1, f_out]]))
        for kc in range(KC):
            nc.sync.dma_start(
                rhsW[:, kc * FO2 : kc * FO2 + f_out],
                w_ap[kc * 128 : (kc + 1) * 128, :],
            )
        # w1 = W @ a1, w2 = W @ a2 appended as columns of rhsW
        # (NOTE tensor_tensor_reduce crashes the device — use scalar_tensor_tensor)
        for kc in range(KC):
            for ai, ab in ((0, a1b), (1, a2b)):
                nc.vector.scalar_tensor_tensor(
                    out=scratch,
                    in0=rhsW[:, kc * FO2 : kc * FO2 + f_out],
                    scalar=1.0,
                    in1=ab,
                    op0=OP.mult,
                    op1=OP.mult,
                    accum_out=rhsW[:, kc * FO2 + f_out + ai : kc * FO2 + f_out + ai + 1],
                )
        rhsW16 = singles.tile([128, KC * FO2], p_dt)
        # w1/w2 broadcast across partitions ([128, k] each) lets s1 (cast
        # bias) and the first chunks' s2 be computed straight from the
        # arriving x tile with multiply+accumulate ops — skipping the
        # transpose->matmul chain that otherwise gates the whole ramp
        w12bc = singles.tile([128, 2 * f_in], F32)
        wrow16 = singles.tile([1, 2 * f_in], p_dt)
        s1raw = singles.tile([128, SUB], F32)
        jd = singles.tile([128, f_in], F32)
        jp = singles.tile([128, f_in], F32)

        with tc.tile_pool(name="wprep", bufs=1, space="PSUM") as wpp:
            wb = wpp.tile([1, 2 * f_in], F32, tag="wb")
            for ai in range(2):
                for kc in range(KC):
                    nc.tensor.transpose(
                        wb[:, ai * f_in + kc * 128 : ai * f_in + (kc + 1) * 128],
                        rhsW[:, kc * FO2 + f_out + ai : kc * FO2 + f_out + ai + 1],
                        ident32,
                    )
            nc.vector.tensor_copy(wrow16, wb)
            wbc = wpp.tile([128, 2 * f_in], F32, tag="wbc")
            for ai in range(2):
                nc.tensor.matmul(
                    wbc[:, ai * f_in : (ai + 1) * f_in],
                    lhsT=ones128[:1, :],
                    rhs=wrow16[:, ai * f_in : (ai + 1) * f_in],
                    start=True,
                    stop=True,
                )
            nc.vector.tensor_copy(w12bc, wbc)

        acc_pool = ctx.enter_context(tc.tile_pool(name="acc", bufs=1, space="PSUM"))
        acc_ps = [
            acc_pool.tile([128, I_BLK], F32, name=f"acc{ih}", tag=f"acc{ih}")
            for ih in range(NIH)
        ]
        # both rowsum accumulators share one PSUM bank at partition
        # offsets 0 and 64 (legal matmul tile positions for M=1)
        rs_bank = acc_pool.tile([128, I_BLK], F32, name="rs_bank", tag="rs_bank")
        rs_ps = [rs_bank[64 * ih : 64 * ih + 1, :] for ih in range(NIH)]

        with ExitStack() as bctx:
            xpool = bctx.enter_context(tc.tile_pool(name="xpool", bufs=x_bufs))
            xtp = bctx.enter_context(tc.tile_pool(name="xtp", bufs=xt_bufs))
            pa_ps = bctx.enter_context(tc.tile_pool(name="pa_ps", bufs=pa_bufs, space="PSUM"))
            tqp = bctx.enter_context(tc.tile_pool(name="tqp", bufs=tq_bufs, space="PSUM"))
            adjp = bctx.enter_context(tc.tile_pool(name="adjp", bufs=adj_bufs))
            zmp = bctx.enter_context(tc.tile_pool(name="zmp", bufs=zm_bufs))
            ptp = bctx.enter_context(tc.tile_pool(name="ptp", bufs=pt_bufs))
            ep = bctx.enter_context(tc.tile_pool(name="ep", bufs=ep_bufs))

            nc.vector.tensor_copy(rhsW16, rhsW)

            xq_tiles = {}

            def emit_xdma(q):
                xbt = xpool.tile([128, XCH * f_in], F32, tag="xbt")
                nc.sync.dma_start(
                    xbt,
                    dram3(
                        x_ap,
                        q * XCH * 128 * f_in,
                        [[f_in, 128], [128 * f_in, XCH], [1, f_in]],
                    ),
                )
                xq_tiles[q] = xbt

            def emit_A_slice(a):
                """Two x chunks (2a, 2a+1): fp32 transposes, one fp16 staging
                copy, h matmuls, h/s2 (and s1-BIG for own rows) stashes."""
                xbt = xq_tiles[a // (XCH // 2)]
                pr = a % (XCH // 2)
                ic0 = 2 * a
                own = ic0 < SUB
                tp = pa_ps.tile([128, 2 * f_in], F32, tag="pa")
                for cc in range(2):
                    c = 2 * pr + cc
                    for kc in range(KC):
                        nc.tensor.transpose(
                            tp[:, cc * f_in + kc * 128 : cc * f_in + kc * 128 + 128],
                            xbt[:, c * f_in + kc * 128 : c * f_in + (kc + 1) * 128],
                            ident32,
                        )
                xT2 = xtp.tile([128, 2 * f_in], p_dt, tag="xT")
                nc.vector.tensor_copy(xT2, tp)
                hps_full = pa_ps.tile([128, 2 * f_in], F32, tag="pa", name=f"hps_{a}")
                hps = hps_full[:, : 2 * FO2]
                for cc in range(2):
                    for kc in range(KC):
                        nc.tensor.matmul(
                            hps[:, cc * FO2 : (cc + 1) * FO2],
                            lhsT=xT2[:, cc * f_in + kc * 128 : cc * f_in + (kc + 1) * 128],
                            rhs=rhsW16[:, kc * FO2 : (kc + 1) * FO2],
                            start=(kc == 0),
                            stop=(kc == KC - 1),
                        )
                h2 = hps.rearrange("p (c f) -> p c f", c=2)
                nc.vector.tensor_copy(
                    h_sb[:, ic0 * f_out : (ic0 + 2) * f_out].rearrange(
                        "p (c f) -> p c f", c=2
                    ),
                    h2[:, :, :f_out],
                )
                if not own:
                    # own chunks' s2 (and s1) come from the ramp-time
                    # multiply+accumulate path instead. The copy lives on ACT:
                    # it precedes its consumer exps in ACT's own in-order
                    # queue, so it can never gate them from another engine.
                    nc.vector.tensor_copy(
                        s2st[:, ic0 : ic0 + 2].rearrange("p (c f) -> p c f", c=2),
                        h2[:, :, f_out + 1 : f_out + 2],
                    )

            def emit_s12():
                """s1 (cast bias) and own-chunk s2 directly from x block 0:
                accum_out of x*w_bc sums over k. DVE takes s1 (it gates every
                cast), Pool takes s2 (only chunk c's prelu needs col c)."""
                xbt = xq_tiles[0]
                for c in range(SUB):
                    xs = xbt[:, c * f_in : (c + 1) * f_in]
                    nc.vector.scalar_tensor_tensor(
                        out=jd, in0=xs, scalar=1.0, in1=w12bc[:, :f_in],
                        op0=OP.mult, op1=OP.mult,
                        accum_out=s1raw[:, c : c + 1],
                    )
                    nc.vector.scalar_tensor_tensor(
                        out=jp, in0=xs, scalar=1.0, in1=w12bc[:, f_in:],
                        op0=OP.mult, op1=OP.mult,
                        accum_out=s2st[:, c : c + 1],
                    )
                    if c % 4 == 3:
                        nc.vector.tensor_scalar(
                            out=s1m[:, c - 3 : c + 1], in0=s1raw[:, c - 3 : c + 1],
                            scalar1=-MASK_BIG, scalar2=None,
                            op0=OP.add, op1=OP.bypass,
                        )

            # adj DMA granules: always 1MB ([128, R, jb] with R*jb == 2048),
            # viewed at the block's chunk width
            adj_views = {}

            def emit_adj(b, d):
                cpj = BLKS[b]
                jb = 128 * cpj
                nd = max(1, cpj // 2)
                R = 8 // nd
                t = adjp.tile([128, 2048], I32, tag="adj", name=f"adj_{b}_{d}")
                v = t.rearrange("p (r j) -> p r j", r=R)
                nc.sync.dma_start(
                    v,
                    dram3(
                        adj_ap,
                        CH0[b] * 128 + d * R * 128 * n,
                        [[n, 128], [128 * n, R], [1, jb]],
                    ),
                )
                adj_views.setdefault(b, {})[d] = v

            # zm tiles pack ceil(1024/jb) s-subtiles per [128, 1024] buffer.
            # The engine routing is a FIXED function of the tile index so each
            # engine recycles its own zm slot ring (tag per engine) — slot
            # WAW reuse then never couples one engine's queue to another's.
            zm_tiles = {}

            def emit_cast(b, s):
                cpj = BLKS[b]
                jb = 128 * cpj
                spt = max(1, 1024 // jb)      # s-subtiles packed per zm tile
                nd = max(1, cpj // 2)
                R = 8 // nd
                tl = zm_tiles.setdefault(b, {})
                ti = s // spt
                if spt == 1:
                    eng = cast_split[ti % len(cast_split)]
                    tag = f"zm{eng}"
                else:
                    # startup small blocks: dedicated ring, engines spread
                    eng = "ppadppad"[s % 8]
                    tag = "zms"
                if ti not in tl:
                    tl[ti] = zmp.tile(
                        [128, 1024], p_dt, tag=tag, name=f"zm_{b}_{ti}",
                        bufs=zm_ring[tag],
                    )
                zm = tl[ti][:, (s % spt) * jb : (s % spt + 1) * jb]
                asl = adj_views[b][s // R][:, s % R, :]
                if eng == "a":
                    # Prelu with alpha=1 == identity affine with ptr bias
                    nc.scalar.activation(
                        out=zm, in_=asl, func=AF.Prelu,
                        bias=s1m[:, s : s + 1], scale=MASK_BIG, alpha=1.0,
                    )
                elif eng == "d":
                    nc.vector.tensor_scalar(
                        out=zm, in0=asl, scalar1=MASK_BIG,
                        scalar2=s1m[:, s : s + 1], op0=OP.mult, op1=OP.add,
                    )
                else:
                    nc.gpsimd.tensor_scalar(
                        out=zm, in0=asl, scalar1=MASK_BIG,
                        scalar2=s1m[:, s : s + 1], op0=OP.mult, op1=OP.add,
                    )

            ucount = [0]
            mm_pending = []

            def emit_group_matmuls(g0, pt2):
                pt3 = pt2.rearrange("p (t n) -> p t n", t=EG)
                for ih in range(NIH):
                    rsl = pt3[:, :, ih * I_BLK : (ih + 1) * I_BLK]
                    for t in range(EG):
                        nc.tensor.matmul(
                            acc_ps[ih],
                            lhsT=h_sb[:, (g0 + t) * f_out : (g0 + t + 1) * f_out],
                            rhs=rsl[:, t, :],
                            start=(g0 == 0 and t == 0),
                            stop=(g0 == NCH - EG and t == EG - 1),
                            skip_group_check=True,
                        )
                        nc.tensor.matmul(
                            rs_ps[ih],
                            lhsT=ones128[:, :1],
                            rhs=rsl[:, t, :],
                            start=(g0 == 0 and t == 0),
                            stop=(g0 == NCH - EG and t == EG - 1),
                            skip_group_check=True,
                        )

            tq_by_chunk = {}

            def emit_T(b, c):
                """PE transposes of one chunk into a tq PSUM tile; runs
                tq_lead chunks ahead of the prelu/exp consumers so ACT never
                waits on PE at block boundaries."""
                cpj = BLKS[b]
                jb = 128 * cpj
                spt = max(1, 1024 // jb)
                tq_t = tqp.tile([128, rows], p_dt, tag="tq", name=f"tq_{b}_{c}")
                for s in range(SUB):
                    nc.tensor.transpose(
                        tq_t[:, s * 128 : (s + 1) * 128],
                        zm_tiles[b][s // spt][
                            :, (s % spt) * jb + c * 128 : (s % spt) * jb + (c + 1) * 128
                        ],
                        identp,
                    )
                tq_by_chunk[CH0[b] + c] = tq_t

            def emit_PX(b, c0):
                """exp(lrelu(z)) == max(e^z, (e^z)^0.2) since exp is
                monotone: ONE ACT op (Exp with the s2 ptr bias, read straight
                from PSUM tq), then a fast-mode pow and a tensor max on DVE.
                Then the (delayed) matmuls of an earlier group."""
                pt2 = ptp.tile([128, EG * rows], p_dt, tag="pt")
                for c in range(c0, c0 + EG):
                    g = CH0[b] + c
                    tq_t = tq_by_chunk.pop(g)
                    s2ptr = s2st[:, g : g + 1]
                    use_dve = dve_every > 0 and (ucount[0] % dve_every == 0)
                    ucount[0] += 1
                    l_t = ep.tile([128, rows], p_dt, tag="l")
                    if use_dve:
                        z_t = ep.tile([128, rows], p_dt, tag="z")
                        nc.vector.tensor_scalar(
                            out=z_t, in0=tq_t, scalar1=s2ptr,
                            scalar2=None, op0=OP.add, op1=OP.bypass,
                        )
                        nc.vector.scalar_tensor_tensor(
                            out=l_t, in0=z_t, scalar=NEG_SLOPE, in1=z_t,
                            op0=OP.mult, op1=OP.max,
                        )
                    else:
                        nc.scalar.activation(
                            out=l_t, in_=tq_t, func=AF.Prelu,
                            bias=s2ptr, scale=1.0, alpha=NEG_SLOPE,
                        )
                    nc.scalar.activation(
                        out=pt2[:, (c - c0) * rows : (c - c0 + 1) * rows],
                        in_=l_t,
                        func=AF.Exp,
                    )
                if len(mm_pending) >= mm_delay:
                    emit_group_matmuls(*mm_pending.pop(0))
                mm_pending.append((CH0[b] + c0, pt2))

            # ---- deadline-sorted emission: each producer unit is emitted
            # when the E cursor (in global chunks) reaches its due chunk, so
            # every in-order engine queue sees stages in data-arrival order ----
            units = []
            xdue = {}
            for q in range(n // (XCH * 128)):
                # front-loaded: all x lands in the first ~16 E chunks, where
                # compute is DMA-bound and idle. Cadence 4 chunks so a parked
                # x DMA (xbt slot reuse) never starves adj on the SP queue.
                due = -100 if q == 0 else 4 * q - la_x
                xdue[q] = due
                units.append((due, 0, "x", q))
            for b, cpj in enumerate(BLKS):
                nd = max(1, cpj // 2)
                for d in range(nd):
                    # last granule of block b lands la_adj chunks before the
                    # block's E groups start (E needs the full column block)
                    units.append(
                        (CH0[b] - la_adj + 2 * (d + 1 - nd), 1, "adj", (b, d))
                    )
            units.append((-99.5, 2, "s12", None))
            for a in range(NCH // 2):
                # front-loaded like x: A-slices chew through the early
                # DMA-bound idle so the drain has no A work left
                due = -99 + a if a < SUB // 2 else max(
                    2 * a + 1 - la_a, xdue[a // (XCH // 2)] + 0.5
                )
                units.append((due, 2, "A", a))
            for b, cpj in enumerate(BLKS):
                nd = max(1, cpj // 2)
                R = SUB // nd
                for s in range(SUB):
                    # 1-chunk spacing; the LAST cast of block b is emitted
                    # la_cast chunks before the block's first E group (which
                    # needs all 8 casts: each chunk's transposes touch every
                    # zm row-tile). Never before its own adj granule.
                    adj_due = CH0[b] - la_adj + 2 * (s // R + 1 - nd)
                    units.append(
                        (
                            max(CH0[b] - la_cast - (SUB - 1 - s), adj_due + 0.5),
                            3,
                            "cast",
                            (b, s),
                        )
                    )
            units.sort(key=lambda u: (u[0], u[1]))

            ui = 0

            def drain_units(e):
                nonlocal ui
                while ui < len(units) and units[ui][0] <= e:
                    _, _, kind, payload = units[ui]
                    ui += 1
                    if kind == "x":
                        emit_xdma(payload)
                    elif kind == "adj":
                        emit_adj(*payload)
                    elif kind == "A":
                        emit_A_slice(payload)
                    elif kind == "s12":
                        emit_s12()
                    else:
                        emit_cast(*payload)

            def chunk_to_bc(g):
                for b in range(len(BLKS)):
                    if CH0[b] <= g < CH0[b + 1]:
                        return b, g - CH0[b]
                return None

            e = 0
            tcur = 0    # transpose cursor (global chunks)
            for b, cpj in enumerate(BLKS):
                for c0 in range(0, cpj, EG):
                    drain_units(e)
                    while tcur < min(e + EG + tq_lead, NCH):
                        emit_T(*chunk_to_bc(tcur))
                        tcur += 1
                    emit_PX(b, c0)
                    e += EG
            drain_units(10**9)
            while mm_pending:
                emit_group_matmuls(*mm_pending.pop(0))

        # ---- phase C: normalize + elu in h'^T space (big [128, I_BLK] ops,
        # rowsum broadcast across partitions by a rank-1 ones matmul), then
        # transpose + store ----
        with ExitStack() as cctx:
            fpool = cctx.enter_context(tc.tile_pool(name="fpool", bufs=2))
            fps = cctx.enter_context(tc.tile_pool(name="fps", bufs=2, space="PSUM"))
            NSUB = I_BLK // 128
            for ih in range(NIH):
                rinv1 = fpool.tile([1, I_BLK], F32, tag="rinv1")
                nc.vector.reciprocal(rinv1, rs_ps[ih])
                rinv16 = fpool.tile([1, I_BLK], p_dt, tag="rinv16")
                nc.vector.tensor_copy(rinv16, rinv1)
                rinv_ps = fps.tile([128, I_BLK], F32, tag="bc")
                nc.tensor.matmul(
                    rinv_ps, lhsT=ones128[:1, :], rhs=rinv16,
                    start=True, stop=True,
                )
                # t1/t2 read acc from PSUM, so the broadcast reciprocal must
                # come from SBUF (one PSUM operand per instruction)
                rinv = fpool.tile([128, I_BLK], F32, tag="rinv")
                nc.vector.tensor_copy(rinv, rinv_ps)
                # elu(v), v = acc/rowsum: relu(v) + exp(min(v, 0)) - 1,
                # with relu(v) = (acc max 0) * rinv and min(v,0) = (acc min 0) * rinv
                t1 = fpool.tile([128, I_BLK], F32, tag="t1")
                nc.vector.scalar_tensor_tensor(
                    out=t1, in0=acc_ps[ih], scalar=0.0, in1=rinv,
                    op0=OP.max, op1=OP.mult,
                )
                t2 = fpool.tile([128, I_BLK], F32, tag="t2")
                nc.vector.scalar_tensor_tensor(
                    out=t2, in0=acc_ps[ih], scalar=0.0, in1=rinv,
                    op0=OP.min, op1=OP.mult,
                )
                t3 = fpool.tile([128, I_BLK], F32, tag="t3")
                nc.scalar.activation(out=t3, in_=t2, func=AF.Exp)
                o_t = fpool.tile([128, I_BLK], F32, tag="o")
                nc.vector.scalar_tensor_tensor(
                    out=o_t, in0=t3, scalar=-1.0, in1=t1, op0=OP.add, op1=OP.add
                )
                tp = fps.tile([128, I_BLK], F32, tag="fps")
                for s in range(NSUB):
                    nc.tensor.transpose(
                        tp[:, s * 128 : (s + 1) * 128],
                        o_t[:, s * 128 : (s + 1) * 128],
                        ident32,
                    )
                o_sb = fpool.tile([128, I_BLK], F32, tag="osb")
                nc.vector.tensor_copy(o_sb, tp)
                nc.scalar.dma_start(
                    dram3(
                        out_ap, ih * I_BLK * f_out,
                        [[f_out, 128], [128 * f_out, NSUB], [1, f_out]],
                    ),
                    o_sb.rearrange("p (s f) -> p s f", s=NSUB),
                )

    nc.compile()
    return nc


_CACHE = {}


def _compiled_full():
    if "nc" not in _CACHE:
        _CACHE["nc"] = build_gat()
    return _CACHE["nc"]


def make_in_maps(x, W, a, adj):
    rows = N_FULL // N_CORES
    in_maps = []
    for c in range(N_CORES):
        sl = slice(c * rows, (c + 1) * rows)
        in_maps.append(
            {
                "x": np.ascontiguousarray(np.roll(x, -c * rows, axis=0)),
                "w": W,
                "a": a,
                "adj": np.ascontiguousarray(np.roll(adj[sl], -c * rows, axis=1)),
            }
        )
    return in_maps


def kernel(x, W, a, adj):
    from concourse.bass_utils import run_bass_kernel_spmd

    nc = _compiled_full()
    x = np.ascontiguousarray(np.asarray(x, dtype=np.float32))
    W = np.ascontiguousarray(np.asarray(W, dtype=np.float32))
    a = np.ascontiguousarray(np.asarray(a, dtype=np.float32))
    adj = np.asarray(adj)
    assert adj.dtype == np.int32
    in_maps = make_in_maps(x, W, a, adj)
    res = run_bass_kernel_spmd(nc, in_maps, core_ids=list(range(N_CORES)))
    out = np.concatenate([res.results[c]["out"] for c in range(N_CORES)], axis=0)
    return out.astype(np.float32)


# revision 55
# speedup vs baseline: 1.0078x; 1.0078x over previous
"""GATv2 layer kernel for Trainium2 — 8 NeuronCores, SPMD row-sharded.

Math (reference):
    h = x @ W
    s1 = h @ a[:F];  s2 = h @ a[F:]
    e  = leaky_relu(s1[:,None] + s2[None,:], 0.2)
    e  = where(adj > 0, e, -9e15)
    att = softmax(e, axis=1)
    out = elu(att @ h)

Kernel strategy (per core, rows of adj/out sharded across 8 cores; x and
adj columns are rotated per core on the host so each core's own rows are
always chunks 0..SUB-1 — one SPMD program, no separate xs input):
  - s1/s2 are separable: s1 = x @ (W @ a1), s2 = x @ (W @ a2); each core
    computes full h (fp16) from the replicated (rotated) x.
  - exponents are tiny (|s1+s2| <~ 5) so softmax needs no max-subtraction:
    P = adj * exp(lrelu(z)) realized as exp(lrelu(z + adjL)) with
    adjL = (adj-1)*60000 (exp of ~-1.2e4 underflows to exactly 0).
  - the int32->fp16 cast of adj folds BOTH the mask affine and the s1 bias
    in one gpsimd pass: zm = adj*BIG + (s1 - BIG)  (per-partition ptr bias).
  - zm tiles are PE-transposed to [j, i] layout BEFORE the nonlinearity, so
    lrelu(+s2)/exp run in transposed orientation and exp writes the
    attention-matmul rhs (P^T) straight to SBUF — no PSUM->SBUF copy pass.
  - softmax row sums come from a ones-vector matmul accumulated in PSUM
    alongside the attention matmul (both PSUM rows share one bank).
  - j blocks have VARIABLE width ([2,2,4] chunks then 8-chunk blocks): the
    first exp group only needs a 2-chunk column block, so the ACT engine
    starts ~8us earlier than with uniform 8-chunk blocks.
  - all input DMAs are issued from the SP queue in exact consumption order
    (x block 0, adj block 0, ...); emission is deadline-sorted so every
    engine's in-order queue sees pipeline stages in data-arrival order.
  - final: normalize + elu in h'^T orientation as a few big [128, 512] ops
    (rowsum reciprocal is broadcast across partitions with a rank-1 ones
    matmul), then transpose and stream to DRAM.
"""

import sys

if "/opt/trn_rl_repo" not in sys.path:
    sys.path.insert(0, "/opt/trn_rl_repo")

from contextlib import ExitStack

import numpy as np

import concourse.bass as bass
import concourse.tile as tile
from concourse import bacc, mybir
from concourse.masks import make_identity

F32 = mybir.dt.float32
F16 = mybir.dt.float16
I32 = mybir.dt.int32
AF = mybir.ActivationFunctionType
OP = mybir.AluOpType

N_FULL = 8192
F_IN = 256
F_OUT = 128
N_CORES = 8
NEG_SLOPE = 0.2
MASK_BIG = 60000.0  # exactly representable in fp16; exp(-0.2*60000) == 0


def build_gat(
    n=N_FULL,
    rows=N_FULL // N_CORES,
    f_in=F_IN,
    f_out=F_OUT,
    blks=(2, 2, 4, 8, 8, 8, 8, 8, 8, 8),   # chunks per j block
    dve_every=2,     # every k-th chunk routes lrelu to DVE instead of ACT
    cast_split="ppdpppdp",  # cast engine by row-tile index (p=Pool, a=ACT, d=DVE)
    zm_ring={"zmp": 12, "zma": 0, "zmd": 4, "zms": 4},
    p_dt=F16,
    adj_bufs=6,
    zm_bufs=20,
    pt_bufs=7,
    ep_bufs=5,
    tq_bufs=3,
    pa_bufs=2,
    x_bufs=5,
    xt_bufs=4,
    eg=2,
    la_x=12,         # emission lookaheads, in global 128-col chunks
    la_adj=8,
    la_a=26,
    la_cast=6,
    mm_delay=3,      # groups of slack between exp and its attention matmuls
    tq_lead=1,       # extra chunks the tq transposes run ahead of prelu/exp
):
    """Build the per-core Bass program. All cores run the identical program;
    per-core behavior comes only from per-core input data (adj shard + the
    host-side rotation of x / adj columns). Returns the compiled module."""
    KC = f_in // 128          # k chunks of f_in
    NCH = n // 128            # column chunks of adj / row chunks of h
    SUB = rows // 128         # i subtiles per core
    I_BLK = min(512, rows)
    NIH = rows // I_BLK       # i halves for matmul psum banks
    FO2 = f_out + 2           # h columns + [s1 s2]
    EG = eg                   # chunks per exp/matmul-delay group
    XCH = 8                   # chunks per x DMA block
    BLKS = list(blks)
    assert sum(BLKS) == NCH and all(c % EG == 0 for c in BLKS)
    CH0 = [0]
    for c in BLKS:
        CH0.append(CH0[-1] + c)

    nc = bacc.Bacc(
        "TRN2",
        target_bir_lowering=False,
        debug=False,
        enable_asserts=False,
        num_devices=1,
    )
    x_ap = nc.dram_tensor("x", [n, f_in], F32, kind="ExternalInput").ap()
    w_ap = nc.dram_tensor("w", [f_in, f_out], F32, kind="ExternalInput").ap()
    a_ap = nc.dram_tensor("a", [2 * f_out, 1], F32, kind="ExternalInput").ap()
    adj_ap = nc.dram_tensor("adj", [rows, n], I32, kind="ExternalInput").ap()
    out_ap = nc.dram_tensor("out", [rows, f_out], F32, kind="ExternalOutput").ap()

    def dram3(ap, off, dims):
        return bass.AP(tensor=ap.tensor, offset=ap.offset + off, ap=dims)

    with tile.TileContext(nc) as tc, ExitStack() as ctx:
        singles = ctx.enter_context(tc.tile_pool(name="singles", bufs=1))

        rhsW = singles.tile([128, KC * FO2], F32)   # per kc: [W chunk | w1 w2]
        ident32 = singles.tile([128, 128], F32)
        make_identity(nc, ident32)
        identp = singles.tile([128, 128], p_dt)
        make_identity(nc, identp)
        h_sb = singles.tile([128, NCH * f_out], p_dt)
        s2st = singles.tile([128, NCH], F32)     # s2[j] in [j%128, j//128]
        s1m = singles.tile([128, SUB], F32)      # s1 - BIG (cast bias ptr)
        ones128 = singles.tile([128, 128], p_dt)
        scratch = singles.tile([128, f_out], F32)
        a1b = singles.tile([128, f_out], F32)
        a2b = singles.tile([128, f_out], F32)

        # ---- constants: rhsW leads the SP DMA queue (everything chains off
        # rhsW16 -> h -> s1m -> casts); a1b/a2b arrive in parallel on ACT ----
        nc.gpsimd.memset(ones128, 1.0)
        nc.scalar.dma_start(a1b, dram3(a_ap, 0, [[0, 128], [1, f_out]]))
        nc.scalar.dma_start(a2b, dram3(a_ap, f_out, [[0, 128], [1, f_out]]))
        for kc in range(KC):
            nc.sync.dma_start(
                rhsW[:, kc * FO2 : kc * FO2 + f_out],
                w_ap[kc * 128 : (kc + 1) * 128, :],
            )
        # w1 = W @ a1, w2 = W @ a2 appended as columns of rhsW
        # (NOTE tensor_tensor_reduce crashes the device — use scalar_tensor_tensor)
        for kc in range(KC):
            for ai, ab in ((0, a1b), (1, a2b)):
                nc.vector.scalar_tensor_tensor(
                    out=scratch,
                    in0=rhsW[:, kc * FO2 : kc * FO2 + f_out],
                    scalar=1.0,
                    in1=ab,
                    op0=OP.mult,
                    op1=OP.mult,
                    accum_out=rhsW[:, kc * FO2 + f_out + ai : kc * FO2 + f_out + ai + 1],
                )
        rhsW16 = singles.tile([128, KC * FO2], p_dt)
        # w1/w2 broadcast across partitions ([128, k] each) lets s1 (cast
        # bias) and the first chunks' s2 be computed straight from the
        # arriving x tile with multiply+accumulate ops — skipping the
        # transpose->matmul chain that otherwise gates the whole ramp
        w12bc = singles.tile([128, 2 * f_in], F32)
        wrow16 = singles.tile([1, 2 * f_in], p_dt)
        s1raw = singles.tile([128, SUB], F32)
        jd = singles.tile([128, f_in], F32)
        jp = singles.tile([128, f_in], F32)

        with tc.tile_pool(name="wprep", bufs=1, space="PSUM") as wpp:
            wb = wpp.tile([1, 2 * f_in], F32, tag="wb")
            for ai in range(2):
                for kc in range(KC):
                    nc.tensor.transpose(
                        wb[:, ai * f_in + kc * 128 : ai * f_in + (kc + 1) * 128],
                        rhsW[:, kc * FO2 + f_out + ai : kc * FO2 + f_out + ai + 1],
                        ident32,
                    )
            nc.vector.tensor_copy(wrow16, wb)
            wbc = wpp.tile([128, 2 * f_in], F32, tag="wbc")
            for ai in range(2):
                nc.tensor.matmul(
                    wbc[:, ai * f_in : (ai + 1) * f_in],
                    lhsT=ones128[:1, :],
                    rhs=wrow16[:, ai * f_in : (ai + 1) * f_in],
                    start=True,
                    stop=True,
                )
            nc.vector.tensor_copy(w12bc, wbc)

        acc_pool = ctx.enter_context(tc.tile_pool(name="acc", bufs=1, space="PSUM"))
        acc_ps = [
            acc_pool.tile([128, I_BLK], F32, name=f"acc{ih}", tag=f"acc{ih}")
            for ih in range(NIH)
        ]
        # both rowsum accumulators share one PSUM bank at partition
        # offsets 0 and 64 (legal matmul tile positions for M=1)
        rs_bank = acc_pool.tile([128, I_BLK], F32, name="rs_bank", tag="rs_bank")
        rs_ps = [rs_bank[64 * ih : 64 * ih + 1, :] for ih in range(NIH)]

        with ExitStack() as bctx:
            xpool = bctx.enter_context(tc.tile_pool(name="xpool", bufs=x_bufs))
            xtp = bctx.enter_context(tc.tile_pool(name="xtp", bufs=xt_bufs))
            pa_ps = bctx.enter_context(tc.tile_pool(name="pa_ps", bufs=pa_bufs, space="PSUM"))
            tqp = bctx.enter_context(tc.tile_pool(name="tqp", bufs=tq_bufs, space="PSUM"))
            adjp = bctx.enter_context(tc.tile_pool(name="adjp", bufs=adj_bufs))
            zmp = bctx.enter_context(tc.tile_pool(name="zmp", bufs=zm_bufs))
            ptp = bctx.enter_context(tc.tile_pool(name="ptp", bufs=pt_bufs))
            ep = bctx.enter_context(tc.tile_pool(name="ep", bufs=ep_bufs))

            nc.vector.tensor_copy(rhsW16, rhsW)

            xq_tiles = {}

            def emit_xdma(q):
                xbt = xpool.tile([128, XCH * f_in], F32, tag="xbt")
                nc.sync.dma_start(
                    xbt,
                    dram3(
                        x_ap,
                        q * XCH * 128 * f_in,
                        [[f_in, 128], [128 * f_in, XCH], [1, f_in]],
                    ),
                )
                xq_tiles[q] = xbt

            def emit_A_slice(a):
                """Two x chunks (2a, 2a+1): fp32 transposes, one fp16 staging
                copy, h matmuls, h/s2 (and s1-BIG for own rows) stashes."""
                xbt = xq_tiles[a // (XCH // 2)]
                pr = a % (XCH // 2)
                ic0 = 2 * a
                own = ic0 < SUB
                tp = pa_ps.tile([128, 2 * f_in], F32, tag="pa")
                for cc in range(2):
                    c = 2 * pr + cc
                    for kc in range(KC):
                        nc.tensor.transpose(
                            tp[:, cc * f_in + kc * 128 : cc * f_in + kc * 128 + 128],
                            xbt[:, c * f_in + kc * 128 : c * f_in + (kc + 1) * 128],
                            ident32,
                        )
                xT2 = xtp.tile([128, 2 * f_in], p_dt, tag="xT")
                nc.vector.tensor_copy(xT2, tp)
                hps_full = pa_ps.tile([128, 2 * f_in], F32, tag="pa", name=f"hps_{a}")
                hps = hps_full[:, : 2 * FO2]
                for cc in range(2):
                    for kc in range(KC):
                        nc.tensor.matmul(
                            hps[:, cc * FO2 : (cc + 1) * FO2],
                            lhsT=xT2[:, cc * f_in + kc * 128 : cc * f_in + (kc + 1) * 128],
                            rhs=rhsW16[:, kc * FO2 : (kc + 1) * FO2],
                            start=(kc == 0),
                            stop=(kc == KC - 1),
                        )
                h2 = hps.rearrange("p (c f) -> p c f", c=2)
                nc.vector.tensor_copy(
                    h_sb[:, ic0 * f_out : (ic0 + 2) * f_out].rearrange(
                        "p (c f) -> p c f", c=2
                    ),
                    h2[:, :, :f_out],
                )
                if not own:
                    # own chunks' s2 (and s1) come from the ramp-time
                    # multiply+accumulate path instead. The copy lives on ACT:
                    # it precedes its consumer exps in ACT's own in-order
                    # queue, so it can never gate them from another engine.
                    nc.vector.tensor_copy(
                        s2st[:, ic0 : ic0 + 2].rearrange("p (c f) -> p c f", c=2),
                        h2[:, :, f_out + 1 : f_out + 2],
                    )

            def emit_s12():
                """s1 (cast bias) and own-chunk s2 directly from x block 0:
                accum_out of x*w_bc sums over k. DVE takes s1 (it gates every
                cast), Pool takes s2 (only chunk c's prelu needs col c)."""
                xbt = xq_tiles[0]
                for c in range(SUB):
                    xs = xbt[:, c * f_in : (c + 1) * f_in]
                    nc.vector.scalar_tensor_tensor(
                        out=jd, in0=xs, scalar=1.0, in1=w12bc[:, :f_in],
                        op0=OP.mult, op1=OP.mult,
                        accum_out=s1raw[:, c : c + 1],
                    )
                    nc.vector.scalar_tensor_tensor(
                        out=jp, in0=xs, scalar=1.0, in1=w12bc[:, f_in:],
                        op0=OP.mult, op1=OP.mult,
                        accum_out=s2st[:, c : c + 1],
                    )
                    if c % 4 == 3:
                        nc.vector.tensor_scalar(
                            out=s1m[:, c - 3 : c + 1], in0=s1raw[:, c - 3 : c + 1],
                            scalar1=-MASK_BIG, scalar2=None,
                            op0=OP.add, op1=OP.bypass,
                        )

            # adj DMA granules: always 1MB ([128, R, jb] with R*jb == 2048),
            # viewed at the block's chunk width
            adj_views = {}

            def emit_adj(b, d):
                cpj = BLKS[b]
                jb = 128 * cpj
                nd = max(1, cpj // 2)
                R = 8 // nd
                t = adjp.tile([128, 2048], I32, tag="adj", name=f"adj_{b}_{d}")
                v = t.rearrange("p (r j) -> p r j", r=R)
                nc.sync.dma_start(
                    v,
                    dram3(
                        adj_ap,
                        CH0[b] * 128 + d * R * 128 * n,
                        [[n, 128], [128 * n, R], [1, jb]],
                    ),
                )
                adj_views.setdefault(b, {})[d] = v

            # zm tiles pack ceil(1024/jb) s-subtiles per [128, 1024] buffer.
            # The engine routing is a FIXED function of the tile index so each
            # engine recycles its own zm slot ring (tag per engine) — slot
            # WAW reuse then never couples one engine's queue to another's.
            zm_tiles = {}

            def emit_cast(b, s):
                cpj = BLKS[b]
                jb = 128 * cpj
                spt = max(1, 1024 // jb)      # s-subtiles packed per zm tile
                nd = max(1, cpj // 2)
                R = 8 // nd
                tl = zm_tiles.setdefault(b, {})
                ti = s // spt
                if spt == 1:
                    eng = cast_split[ti % len(cast_split)]
                    tag = f"zm{eng}"
                else:
                    # startup small blocks: dedicated ring, engines spread
                    eng = "ppadppad"[s % 8]
                    tag = "zms"
                if ti not in tl:
                    tl[ti] = zmp.tile(
                        [128, 1024], p_dt, tag=tag, name=f"zm_{b}_{ti}",
                        bufs=zm_ring[tag],
                    )
                zm = tl[ti][:, (s % spt) * jb : (s % spt + 1) * jb]
                asl = adj_views[b][s // R][:, s % R, :]
                if eng == "a":
                    # Prelu with alpha=1 == identity affine with ptr bias
                    nc.scalar.activation(
                        out=zm, in_=asl, func=AF.Prelu,
                        bias=s1m[:, s : s + 1], scale=MASK_BIG, alpha=1.0,
                    )
                elif eng == "d":
                    nc.vector.tensor_scalar(
                        out=zm, in0=asl, scalar1=MASK_BIG,
                        scalar2=s1m[:, s : s + 1], op0=OP.mult, op1=OP.add,
                    )
                else:
                    nc.gpsimd.tensor_scalar(
                        out=zm, in0=asl, scalar1=MASK_BIG,
                        scalar2=s1m[:, s : s + 1], op0=OP.mult, op1=OP.add,
                    )

            ucount = [0]
            mm_pending = []

            def emit_group_matmuls(g0, pt2):
                pt3 = pt2.rearrange("p (t n) -> p t n", t=EG)
                for ih in range(NIH):
                    rsl = pt3[:, :, ih * I_BLK : (ih + 1) * I_BLK]
                    for t in range(EG):
                        nc.tensor.matmul(
                            acc_ps[ih],
                            lhsT=h_sb[:, (g0 + t) * f_out : (g0 + t + 1) * f_out],
                            rhs=rsl[:, t, :],
                            start=(g0 == 0 and t == 0),
                            stop=(g0 == NCH - EG and t == EG - 1),
                            skip_group_check=True,
                        )
                        nc.tensor.matmul(
                            rs_ps[ih],
                            lhsT=ones128[:, :1],
                            rhs=rsl[:, t, :],
                            start=(g0 == 0 and t == 0),
                            stop=(g0 == NCH - EG and t == EG - 1),
                            skip_group_check=True,
                        )

            tq_by_chunk = {}

            def emit_T(b, c):
                """PE transposes of one chunk into a tq PSUM tile; runs
                tq_lead chunks ahead of the prelu/exp consumers so ACT never
                waits on PE at block boundaries."""
                cpj = BLKS[b]
                jb = 128 * cpj
                spt = max(1, 1024 // jb)
                tq_t = tqp.tile([128, rows], p_dt, tag="tq", name=f"tq_{b}_{c}")
                for s in range(SUB):
                    nc.tensor.transpose(
                        tq_t[:, s * 128 : (s + 1) * 128],
                        zm_tiles[b][s // spt][
                            :, (s % spt) * jb + c * 128 : (s % spt) * jb + (c + 1) * 128
                        ],
                        identp,
                    )
                tq_by_chunk[CH0[b] + c] = tq_t

            def emit_PX(b, c0):
                """exp(lrelu(z)) == max(e^z, (e^z)^0.2) since exp is
                monotone: ONE ACT op (Exp with the s2 ptr bias, read straight
                from PSUM tq), then a fast-mode pow and a tensor max on DVE.
                Then the (delayed) matmuls of an earlier group."""
                pt2 = ptp.tile([128, EG * rows], p_dt, tag="pt")
                for c in range(c0, c0 + EG):
                    g = CH0[b] + c
                    tq_t = tq_by_chunk.pop(g)
                    s2ptr = s2st[:, g : g + 1]
                    use_dve = dve_every > 0 and (ucount[0] % dve_every == 0)
                    ucount[0] += 1
                    l_t = ep.tile([128, rows], p_dt, tag="l")
                    if use_dve:
                        z_t = ep.tile([128, rows], p_dt, tag="z")
                        nc.vector.tensor_scalar(
                            out=z_t, in0=tq_t, scalar1=s2ptr,
                            scalar2=None, op0=OP.add, op1=OP.bypass,
                        )
                        nc.vector.scalar_tensor_tensor(
                            out=l_t, in0=z_t, scalar=NEG_SLOPE, in1=z_t,
                            op0=OP.mult, op1=OP.max,
                        )
                    else:
                        nc.scalar.activation(
                            out=l_t, in_=tq_t, func=AF.Prelu,
                            bias=s2ptr, scale=1.0, alpha=NEG_SLOPE,
                        )
                    nc.scalar.activation(
                        out=pt2[:, (c - c0) * rows : (c - c0 + 1) * rows],
                        in_=l_t,
                        func=AF.Exp,
                    )
                if len(mm_pending) >= mm_delay:
                    emit_group_matmuls(*mm_pending.pop(0))
                mm_pending.append((CH0[b] + c0, pt2))

            # ---- deadline-sorted emission: each producer unit is emitted
            # when the E cursor (in global chunks) reaches its due chunk, so
            # every in-order engine queue sees stages in data-arrival order ----
            units = []
            xdue = {}
            for q in range(n // (XCH * 128)):
                # front-loaded: all x lands in the first ~16 E chunks, where
                # compute is DMA-bound and idle. Cadence 4 chunks so a parked
                # x DMA (xbt slot reuse) never starves adj on the SP queue.
                due = -100 if q == 0 else 4 * q - la_x
                xdue[q] = due
                units.append((due, 0, "x", q))
            for b, cpj in enumerate(BLKS):
                nd = max(1, cpj // 2)
                for d in range(nd):
                    # last granule of block b lands la_adj chunks before the
                    # block's E groups start (E needs the full column block)
                    units.append(
                        (CH0[b] - la_adj + 2 * (d + 1 - nd), 1, "adj", (b, d))
                    )
            units.append((-99.5, 2, "s12", None))
            for a in range(NCH // 2):
                # front-loaded like x: A-slices chew through the early
                # DMA-bound idle so the drain has no A work left
                due = -99 + a if a < SUB // 2 else max(
                    2 * a + 1 - la_a, xdue[a // (XCH // 2)] + 0.5
                )
                units.append((due, 2, "A", a))
            for b, cpj in enumerate(BLKS):
                nd = max(1, cpj // 2)
                R = SUB // nd
                for s in range(SUB):
                    # 1-chunk spacing; the LAST cast of block b is emitted
                    # la_cast chunks before the block's first E group (which
                    # needs all 8 casts: each chunk's transposes touch every
                    # zm row-tile). Never before its own adj granule.
                    adj_due = CH0[b] - la_adj + 2 * (s // R + 1 - nd)
                    units.append(
                        (
                            max(CH0[b] - la_cast - (SUB - 1 - s), adj_due + 0.5),
                            3,
                            "cast",
                            (b, s),
                        )
                    )
            units.sort(key=lambda u: (u[0], u[1]))

            ui = 0

            def drain_units(e):
                nonlocal ui
                while ui < len(units) and units[ui][0] <= e:
                    _, _, kind, payload = units[ui]
                    ui += 1
                    if kind == "x":
                        emit_xdma(payload)
                    elif kind == "adj":
                        emit_adj(*payload)
                    elif kind == "A":
                        emit_A_slice(payload)
                    elif kind == "s12":
                        emit_s12()
                    else:
                        emit_cast(*payload)

            def chunk_to_bc(g):
                for b in range(len(BLKS)):
                    if CH0[b] <= g < CH0[b + 1]:
                        return b, g - CH0[b]
                return None

            e = 0
            tcur = 0    # transpose cursor (global chunks)
            for b, cpj in enumerate(BLKS):
                for c0 in range(0, cpj, EG):
                    drain_units(e)
                    while tcur < min(e + EG + tq_lead, NCH):
                        emit_T(*chunk_to_bc(tcur))
                        tcur += 1
                    emit_PX(b, c0)
                    e += EG
            drain_units(10**9)
            while mm_pending:
                emit_group_matmuls(*mm_pending.pop(0))

        # ---- phase C: normalize + elu in h'^T space (big [128, I_BLK] ops,
        # rowsum broadcast across partitions by a rank-1 ones matmul), then
        # transpose + store ----
        with ExitStack() as cctx:
            fpool = cctx.enter_context(tc.tile_pool(name="fpool", bufs=2))
            fps = cctx.enter_context(tc.tile_pool(name="fps", bufs=2, space="PSUM"))
            NSUB = I_BLK // 128
            for ih in range(NIH):
                rinv1 = fpool.tile([1, I_BLK], F32, tag="rinv1")
                nc.vector.reciprocal(rinv1, rs_ps[ih])
                rinv16 = fpool.tile([1, I_BLK], p_dt, tag="rinv16")
                nc.vector.tensor_copy(rinv16, rinv1)
                rinv_ps = fps.tile([128, I_BLK], F32, tag="bc")
                nc.tensor.matmul(
                    rinv_ps, lhsT=ones128[:1, :], rhs=rinv16,
                    start=True, stop=True,
                )
                # t1/t2 read acc from PSUM, so the broadcast reciprocal must
                # come from SBUF (one PSUM operand per instruction)
                rinv = fpool.tile([128, I_BLK], F32, tag="rinv")
                nc.vector.tensor_copy(rinv, rinv_ps)
                # elu(v), v = acc/rowsum: relu(v) + exp(min(v, 0)) - 1,
                # with relu(v) = (acc max 0) * rinv and min(v,0) = (acc min 0) * rinv
                t1 = fpool.tile([128, I_BLK], F32, tag="t1")
                nc.vector.scalar_tensor_tensor(
                    out=t1, in0=acc_ps[ih], scalar=0.0, in1=rinv,
                    op0=OP.max, op1=OP.mult,
                )
                t2 = fpool.tile([128, I_BLK], F32, tag="t2")
                nc.vector.scalar_tensor_tensor(
                    out=t2, in0=acc_ps[ih], scalar=0.0, in1=rinv,
                    op0=OP.min, op1=OP.mult,
                )
                t3 = fpool.tile([128, I_BLK], F32, tag="t3")
                nc.scalar.activation(out=t3, in_=t2, func=AF.Exp)
                o_t = fpool.tile([128, I_BLK], F32, tag="o")
                nc.vector.scalar_tensor_tensor(
                    out=o_t, in0=t3, scalar=-1.0, in1=t1, op0=OP.add, op1=OP.add
                )
                tp = fps.tile([128, I_BLK], F32, tag="fps")
                for s in range(NSUB):
                    nc.tensor.transpose(
                        tp[:, s * 128 : (s + 1) * 128],
                        o_t[:, s * 128 : (s + 1) * 128],
                        ident32,
                    )
                o_sb = fpool.tile([128, I_BLK], F32, tag="osb")
                nc.vector.tensor_copy(o_sb, tp)
                nc.scalar.dma_start(
                    dram3(
                        out_ap, ih * I_BLK * f_out,
                        [[f_out, 128], [128 * f_out, NSUB], [1, f_out]],
                    ),
                    o_sb.rearrange("p (s f) -> p s f", s=NSUB),
                )

    nc.compile()
    return nc


_CACHE = {}


def _compiled_full():
    if "nc" not in _CACHE:
        _CACHE["nc"] = build_gat()
    return _CACHE["nc"]


def make_in_maps(x, W, a, adj):
    rows = N_FULL // N_CORES
    in_maps = []
    for c in range(N_CORES):
        sl = slice(c * rows, (c + 1) * rows)
        in_maps.append(
            {
                "x": np.ascontiguousarray(np.roll(x, -c * rows, axis=0)),
                "w": W,
                "a": a,
                "adj": np.ascontiguousarray(np.roll(adj[sl], -c * rows, axis=1)),
            }
        )
    return in_maps


def kernel(x, W, a, adj):
    from concourse.bass_utils import run_bass_kernel_spmd

    nc = _compiled_full()
    x = np.ascontiguousarray(np.asarray(x, dtype=np.float32))
    W = np.ascontiguousarray(np.asarray(W, dtype=np.float32))
    a = np.ascontiguousarray(np.asarray(a, dtype=np.float32))
    adj = np.asarray(adj)
    assert adj.dtype == np.int32
    in_maps = make_in_maps(x, W, a, adj)
    res = run_bass_kernel_spmd(nc, in_maps, core_ids=list(range(N_CORES)))
    out = np.concatenate([res.results[c]["out"] for c in range(N_CORES)], axis=0)
    return out.astype(np.float32)


# revision 62
# speedup vs baseline: 1.0194x; 1.0115x over previous
"""GATv2 layer kernel for Trainium2 — 8 NeuronCores, SPMD row-sharded.

Math (reference):
    h = x @ W
    s1 = h @ a[:F];  s2 = h @ a[F:]
    e  = leaky_relu(s1[:,None] + s2[None,:], 0.2)
    e  = where(adj > 0, e, -9e15)
    att = softmax(e, axis=1)
    out = elu(att @ h)

Kernel strategy (per core, rows of adj/out sharded across 8 cores; x and
adj columns are rotated per core on the host so each core's own rows are
always chunks 0..SUB-1 — one SPMD program, no separate xs input):
  - s1/s2 are separable: s1 = x @ (W @ a1), s2 = x @ (W @ a2); each core
    computes full h (fp16) from the replicated (rotated) x.
  - exponents are tiny (|s1+s2| <~ 5) so softmax needs no max-subtraction:
    P = adj * exp(lrelu(z)) realized as exp(lrelu(z + adjL)) with
    adjL = (adj-1)*60000 (exp of ~-1.2e4 underflows to exactly 0).
  - the int32->fp16 cast of adj folds BOTH the mask affine and the s1 bias
    in one gpsimd pass: zm = adj*BIG + (s1 - BIG)  (per-partition ptr bias).
  - zm tiles are PE-transposed to [j, i] layout BEFORE the nonlinearity, so
    lrelu(+s2)/exp run in transposed orientation and exp writes the
    attention-matmul rhs (P^T) straight to SBUF — no PSUM->SBUF copy pass.
  - softmax row sums come from a ones-vector matmul accumulated in PSUM
    alongside the attention matmul (both PSUM rows share one bank).
  - j blocks have VARIABLE width ([2,2,4] chunks then 8-chunk blocks): the
    first exp group only needs a 2-chunk column block, so the ACT engine
    starts ~8us earlier than with uniform 8-chunk blocks.
  - all input DMAs are issued from the SP queue in exact consumption order
    (x block 0, adj block 0, ...); emission is deadline-sorted so every
    engine's in-order queue sees pipeline stages in data-arrival order.
  - final: normalize + elu in h'^T orientation as a few big [128, 512] ops
    (rowsum reciprocal is broadcast across partitions with a rank-1 ones
    matmul), then transpose and stream to DRAM.
"""

import sys

if "/opt/trn_rl_repo" not in sys.path:
    sys.path.insert(0, "/opt/trn_rl_repo")

from contextlib import ExitStack

import numpy as np

import concourse.bass as bass
import concourse.tile as tile
from concourse import bacc, mybir
from concourse.masks import make_identity

F32 = mybir.dt.float32
F16 = mybir.dt.float16
I32 = mybir.dt.int32
AF = mybir.ActivationFunctionType
OP = mybir.AluOpType

N_FULL = 8192
F_IN = 256
F_OUT = 128
N_CORES = 8
NEG_SLOPE = 0.2
MASK_BIG = 60000.0  # exactly representable in fp16; exp(-0.2*60000) == 0


def build_gat(
    n=N_FULL,
    rows=N_FULL // N_CORES,
    f_in=F_IN,
    f_out=F_OUT,
    blks=(2, 2, 4, 8, 8, 8, 8, 8, 8, 8),   # chunks per j block
    dve_every=2,     # every k-th chunk routes lrelu to DVE instead of ACT
    cast_split="ppdpppdp",  # cast engine by row-tile index (p=Pool, a=ACT, d=DVE)
    zm_ring={"zmp": 12, "zma": 0, "zmd": 4, "zms": 4},
    p_dt=F16,
    adj_bufs=6,
    zm_bufs=20,
    pt_bufs=7,
    ep_bufs=5,
    tq_bufs=3,
    pa_bufs=2,
    x_bufs=5,
    xt_bufs=4,
    eg=2,
    la_x=12,         # emission lookaheads, in global 128-col chunks
    la_adj=8,
    la_a=18,
    la_cast=6,
    mm_delay=3,      # groups of slack between exp and its attention matmuls
    tq_lead=1,       # extra chunks the tq transposes run ahead of prelu/exp
    drain_from=52,   # chunk index where drain routing kicks in
    drain_mod=3,     # in drain, route (mod-1)/mod of lrelu groups to DVE
):
    """Build the per-core Bass program. All cores run the identical program;
    per-core behavior comes only from per-core input data (adj shard + the
    host-side rotation of x / adj columns). Returns the compiled module."""
    KC = f_in // 128          # k chunks of f_in
    NCH = n // 128            # column chunks of adj / row chunks of h
    SUB = rows // 128         # i subtiles per core
    I_BLK = min(512, rows)
    NIH = rows // I_BLK       # i halves for matmul psum banks
    FO2 = f_out + 2           # h columns + [s1 s2]
    EG = eg                   # chunks per exp/matmul-delay group
    XCH = 8                   # chunks per x DMA block
    BLKS = list(blks)
    assert sum(BLKS) == NCH and all(c % EG == 0 for c in BLKS)
    CH0 = [0]
    for c in BLKS:
        CH0.append(CH0[-1] + c)

    nc = bacc.Bacc(
        "TRN2",
        target_bir_lowering=False,
        debug=False,
        enable_asserts=False,
        num_devices=1,
    )
    x_ap = nc.dram_tensor("x", [n, f_in], F32, kind="ExternalInput").ap()
    w_ap = nc.dram_tensor("w", [f_in, f_out], F32, kind="ExternalInput").ap()
    a_ap = nc.dram_tensor("a", [2 * f_out, 1], F32, kind="ExternalInput").ap()
    adj_ap = nc.dram_tensor("adj", [rows, n], I32, kind="ExternalInput").ap()
    out_ap = nc.dram_tensor("out", [rows, f_out], F32, kind="ExternalOutput").ap()

    def dram3(ap, off, dims):
        return bass.AP(tensor=ap.tensor, offset=ap.offset + off, ap=dims)

    with tile.TileContext(nc) as tc, ExitStack() as ctx:
        singles = ctx.enter_context(tc.tile_pool(name="singles", bufs=1))

        rhsW = singles.tile([128, KC * FO2], F32)   # per kc: [W chunk | w1 w2]
        ident32 = singles.tile([128, 128], F32)
        make_identity(nc, ident32)
        identp = singles.tile([128, 128], p_dt)
        make_identity(nc, identp)
        h_sb = singles.tile([128, NCH * f_out], p_dt)
        s2st = singles.tile([128, NCH], F32)     # s2[j] in [j%128, j//128]
        s1m = singles.tile([128, SUB], F32)      # s1 - BIG (cast bias ptr)
        ones128 = singles.tile([128, 128], p_dt)
        scratch = singles.tile([128, f_out], F32)
        a1b = singles.tile([128, f_out], F32)
        a2b = singles.tile([128, f_out], F32)

        # ---- constants: rhsW leads the SP DMA queue (everything chains off
        # rhsW16 -> h -> s1m -> casts); a1b/a2b arrive in parallel on ACT ----
        nc.gpsimd.memset(ones128, 1.0)
        nc.scalar.dma_start(a1b, dram3(a_ap, 0, [[0, 128], [1, f_out]]))
        nc.scalar.dma_start(a2b, dram3(a_ap, f_out, [[0, 128], [1, f_out]]))
        for kc in range(KC):
            nc.sync.dma_start(
                rhsW[:, kc * FO2 : kc * FO2 + f_out],
                w_ap[kc * 128 : (kc + 1) * 128, :],
            )
        # w1 = W @ a1, w2 = W @ a2 appended as columns of rhsW
        # (NOTE tensor_tensor_reduce crashes the device — use scalar_tensor_tensor)
        for kc in range(KC):
            for ai, ab in ((0, a1b), (1, a2b)):
                nc.vector.scalar_tensor_tensor(
                    out=scratch,
                    in0=rhsW[:, kc * FO2 : kc * FO2 + f_out],
                    scalar=1.0,
                    in1=ab,
                    op0=OP.mult,
                    op1=OP.mult,
                    accum_out=rhsW[:, kc * FO2 + f_out + ai : kc * FO2 + f_out + ai + 1],
                )
        rhsW16 = singles.tile([128, KC * FO2], p_dt)
        # w1/w2 broadcast across partitions ([128, k] each) lets s1 (cast
        # bias) and the first chunks' s2 be computed straight from the
        # arriving x tile with multiply+accumulate ops — skipping the
        # transpose->matmul chain that otherwise gates the whole ramp
        w12bc = singles.tile([128, 2 * f_in], F32)
        wrow16 = singles.tile([1, 2 * f_in], p_dt)
        s1raw = singles.tile([128, SUB], F32)
        jd = singles.tile([128, f_in], F32)
        jp = singles.tile([128, f_in], F32)

        with tc.tile_pool(name="wprep", bufs=1, space="PSUM") as wpp:
            wb = wpp.tile([1, 2 * f_in], F32, tag="wb")
            for ai in range(2):
                for kc in range(KC):
                    nc.tensor.transpose(
                        wb[:, ai * f_in + kc * 128 : ai * f_in + (kc + 1) * 128],
                        rhsW[:, kc * FO2 + f_out + ai : kc * FO2 + f_out + ai + 1],
                        ident32,
                    )
            nc.vector.tensor_copy(wrow16, wb)
            wbc = wpp.tile([128, 2 * f_in], F32, tag="wbc")
            for ai in range(2):
                nc.tensor.matmul(
                    wbc[:, ai * f_in : (ai + 1) * f_in],
                    lhsT=ones128[:1, :],
                    rhs=wrow16[:, ai * f_in : (ai + 1) * f_in],
                    start=True,
                    stop=True,
                )
            nc.vector.tensor_copy(w12bc, wbc)

        acc_pool = ctx.enter_context(tc.tile_pool(name="acc", bufs=1, space="PSUM"))
        acc_ps = [
            acc_pool.tile([128, I_BLK], F32, name=f"acc{ih}", tag=f"acc{ih}")
            for ih in range(NIH)
        ]
        # both rowsum accumulators share one PSUM bank at partition
        # offsets 0 and 64 (legal matmul tile positions for M=1)
        rs_bank = acc_pool.tile([128, I_BLK], F32, name="rs_bank", tag="rs_bank")
        rs_ps = [rs_bank[64 * ih : 64 * ih + 1, :] for ih in range(NIH)]

        with ExitStack() as bctx:
            xpool = bctx.enter_context(tc.tile_pool(name="xpool", bufs=x_bufs))
            xtp = bctx.enter_context(tc.tile_pool(name="xtp", bufs=xt_bufs))
            pa_ps = bctx.enter_context(tc.tile_pool(name="pa_ps", bufs=pa_bufs, space="PSUM"))
            tqp = bctx.enter_context(tc.tile_pool(name="tqp", bufs=tq_bufs, space="PSUM"))
            adjp = bctx.enter_context(tc.tile_pool(name="adjp", bufs=adj_bufs))
            zmp = bctx.enter_context(tc.tile_pool(name="zmp", bufs=zm_bufs))
            ptp = bctx.enter_context(tc.tile_pool(name="ptp", bufs=pt_bufs))
            ep = bctx.enter_context(tc.tile_pool(name="ep", bufs=ep_bufs))

            nc.vector.tensor_copy(rhsW16, rhsW)

            xq_tiles = {}

            def emit_xdma(q):
                xbt = xpool.tile([128, XCH * f_in], F32, tag="xbt")
                nc.sync.dma_start(
                    xbt,
                    dram3(
                        x_ap,
                        q * XCH * 128 * f_in,
                        [[f_in, 128], [128 * f_in, XCH], [1, f_in]],
                    ),
                )
                xq_tiles[q] = xbt

            def emit_A_slice(a):
                """Two x chunks (2a, 2a+1): fp32 transposes, one fp16 staging
                copy, h matmuls, h/s2 (and s1-BIG for own rows) stashes."""
                xbt = xq_tiles[a // (XCH // 2)]
                pr = a % (XCH // 2)
                ic0 = 2 * a
                own = ic0 < SUB
                tp = pa_ps.tile([128, 2 * f_in], F32, tag="pa")
                for cc in range(2):
                    c = 2 * pr + cc
                    for kc in range(KC):
                        nc.tensor.transpose(
                            tp[:, cc * f_in + kc * 128 : cc * f_in + kc * 128 + 128],
                            xbt[:, c * f_in + kc * 128 : c * f_in + (kc + 1) * 128],
                            ident32,
                        )
                xT2 = xtp.tile([128, 2 * f_in], p_dt, tag="xT")
                nc.vector.tensor_copy(xT2, tp)
                hps_full = pa_ps.tile([128, 2 * f_in], F32, tag="pa", name=f"hps_{a}")
                hps = hps_full[:, : 2 * FO2]
                for cc in range(2):
                    for kc in range(KC):
                        nc.tensor.matmul(
                            hps[:, cc * FO2 : (cc + 1) * FO2],
                            lhsT=xT2[:, cc * f_in + kc * 128 : cc * f_in + (kc + 1) * 128],
                            rhs=rhsW16[:, kc * FO2 : (kc + 1) * FO2],
                            start=(kc == 0),
                            stop=(kc == KC - 1),
                        )
                h2 = hps.rearrange("p (c f) -> p c f", c=2)
                nc.vector.tensor_copy(
                    h_sb[:, ic0 * f_out : (ic0 + 2) * f_out].rearrange(
                        "p (c f) -> p c f", c=2
                    ),
                    h2[:, :, :f_out],
                )
                if not own:
                    # own chunks' s2 (and s1) come from the ramp-time
                    # multiply+accumulate path instead. The copy lives on ACT:
                    # it precedes its consumer exps in ACT's own in-order
                    # queue, so it can never gate them from another engine.
                    nc.vector.tensor_copy(
                        s2st[:, ic0 : ic0 + 2].rearrange("p (c f) -> p c f", c=2),
                        h2[:, :, f_out + 1 : f_out + 2],
                    )

            def emit_s12():
                """s1 (cast bias) and own-chunk s2 directly from x block 0:
                accum_out of x*w_bc sums over k. DVE takes s1 (it gates every
                cast), Pool takes s2 (only chunk c's prelu needs col c)."""
                xbt = xq_tiles[0]
                for c in range(SUB):
                    xs = xbt[:, c * f_in : (c + 1) * f_in]
                    nc.vector.scalar_tensor_tensor(
                        out=jd, in0=xs, scalar=1.0, in1=w12bc[:, :f_in],
                        op0=OP.mult, op1=OP.mult,
                        accum_out=s1raw[:, c : c + 1],
                    )
                    nc.vector.scalar_tensor_tensor(
                        out=jp, in0=xs, scalar=1.0, in1=w12bc[:, f_in:],
                        op0=OP.mult, op1=OP.mult,
                        accum_out=s2st[:, c : c + 1],
                    )
                    if c % 4 == 3:
                        nc.vector.tensor_scalar(
                            out=s1m[:, c - 3 : c + 1], in0=s1raw[:, c - 3 : c + 1],
                            scalar1=-MASK_BIG, scalar2=None,
                            op0=OP.add, op1=OP.bypass,
                        )

            # adj DMA granules: always 1MB ([128, R, jb] with R*jb == 2048),
            # viewed at the block's chunk width
            adj_views = {}

            def emit_adj(b, d):
                cpj = BLKS[b]
                jb = 128 * cpj
                nd = max(1, cpj // 2)
                R = 8 // nd
                t = adjp.tile([128, 2048], I32, tag="adj", name=f"adj_{b}_{d}")
                v = t.rearrange("p (r j) -> p r j", r=R)
                nc.sync.dma_start(
                    v,
                    dram3(
                        adj_ap,
                        CH0[b] * 128 + d * R * 128 * n,
                        [[n, 128], [128 * n, R], [1, jb]],
                    ),
                )
                adj_views.setdefault(b, {})[d] = v

            # zm tiles pack ceil(1024/jb) s-subtiles per [128, 1024] buffer.
            # The engine routing is a FIXED function of the tile index so each
            # engine recycles its own zm slot ring (tag per engine) — slot
            # WAW reuse then never couples one engine's queue to another's.
            zm_tiles = {}

            def emit_cast(b, s):
                cpj = BLKS[b]
                jb = 128 * cpj
                spt = max(1, 1024 // jb)      # s-subtiles packed per zm tile
                nd = max(1, cpj // 2)
                R = 8 // nd
                tl = zm_tiles.setdefault(b, {})
                ti = s // spt
                if spt == 1:
                    eng = cast_split[ti % len(cast_split)]
                    tag = f"zm{eng}"
                else:
                    # startup small blocks: dedicated ring, engines spread
                    eng = "ppadppad"[s % 8]
                    tag = "zms"
                if ti not in tl:
                    tl[ti] = zmp.tile(
                        [128, 1024], p_dt, tag=tag, name=f"zm_{b}_{ti}",
                        bufs=zm_ring[tag],
                    )
                zm = tl[ti][:, (s % spt) * jb : (s % spt + 1) * jb]
                asl = adj_views[b][s // R][:, s % R, :]
                if eng == "a":
                    # Prelu with alpha=1 == identity affine with ptr bias
                    nc.scalar.activation(
                        out=zm, in_=asl, func=AF.Prelu,
                        bias=s1m[:, s : s + 1], scale=MASK_BIG, alpha=1.0,
                    )
                elif eng == "d":
                    nc.vector.tensor_scalar(
                        out=zm, in0=asl, scalar1=MASK_BIG,
                        scalar2=s1m[:, s : s + 1], op0=OP.mult, op1=OP.add,
                    )
                else:
                    nc.gpsimd.tensor_scalar(
                        out=zm, in0=asl, scalar1=MASK_BIG,
                        scalar2=s1m[:, s : s + 1], op0=OP.mult, op1=OP.add,
                    )

            ucount = [0]
            mm_pending = []

            def emit_group_matmuls(g0, pt2):
                pt3 = pt2.rearrange("p (t n) -> p t n", t=EG)
                for ih in range(NIH):
                    rsl = pt3[:, :, ih * I_BLK : (ih + 1) * I_BLK]
                    for t in range(EG):
                        nc.tensor.matmul(
                            acc_ps[ih],
                            lhsT=h_sb[:, (g0 + t) * f_out : (g0 + t + 1) * f_out],
                            rhs=rsl[:, t, :],
                            start=(g0 == 0 and t == 0),
                            stop=(g0 == NCH - EG and t == EG - 1),
                            skip_group_check=True,
                        )
                        nc.tensor.matmul(
                            rs_ps[ih],
                            lhsT=ones128[:, :1],
                            rhs=rsl[:, t, :],
                            start=(g0 == 0 and t == 0),
                            stop=(g0 == NCH - EG and t == EG - 1),
                            skip_group_check=True,
                        )

            tq_by_chunk = {}

            def emit_T(b, c):
                """PE transposes of one chunk into a tq PSUM tile; runs
                tq_lead chunks ahead of the prelu/exp consumers so ACT never
                waits on PE at block boundaries."""
                cpj = BLKS[b]
                jb = 128 * cpj
                spt = max(1, 1024 // jb)
                tq_t = tqp.tile([128, rows], p_dt, tag="tq", name=f"tq_{b}_{c}")
                for s in range(SUB):
                    nc.tensor.transpose(
                        tq_t[:, s * 128 : (s + 1) * 128],
                        zm_tiles[b][s // spt][
                            :, (s % spt) * jb + c * 128 : (s % spt) * jb + (c + 1) * 128
                        ],
                        identp,
                    )
                tq_by_chunk[CH0[b] + c] = tq_t

            def emit_PX(b, c0):
                """exp(lrelu(z)) == max(e^z, (e^z)^0.2) since exp is
                monotone: ONE ACT op (Exp with the s2 ptr bias, read straight
                from PSUM tq), then a fast-mode pow and a tensor max on DVE.
                Then the (delayed) matmuls of an earlier group."""
                pt2 = ptp.tile([128, EG * rows], p_dt, tag="pt")
                l2 = ep.tile([128, EG * rows], p_dt, tag="l", bufs=3)
                # in the drain (A copies done) DVE has slack while ACT
                # saturates: route lrelu to DVE more aggressively there
                k = ucount[0]
                if CH0[b] + c0 >= drain_from:
                    use_dve = k % drain_mod != drain_mod - 1
                else:
                    use_dve = dve_every > 0 and (k % dve_every == 0)
                ucount[0] += 1
                for c in range(c0, c0 + EG):
                    g = CH0[b] + c
                    tq_t = tq_by_chunk.pop(g)
                    s2ptr = s2st[:, g : g + 1]
                    l_t = l2[:, (c - c0) * rows : (c - c0 + 1) * rows]
                    if use_dve:
                        # lrelu via fast-mode ops only: z+s2 (2x, PSUM in),
                        # 0.2*z (4x), tensor max (2x) — the stt form has no
                        # DVE fast mode and costs ~2x more
                        z_t = ep.tile([128, rows], p_dt, tag="z", bufs=2)
                        nc.vector.tensor_scalar(
                            out=z_t, in0=tq_t, scalar1=s2ptr,
                            scalar2=None, op0=OP.add, op1=OP.bypass,
                        )
                        u_t = ep.tile([128, rows], p_dt, tag="u", bufs=2)
                        nc.vector.tensor_scalar(
                            out=u_t, in0=z_t, scalar1=NEG_SLOPE,
                            scalar2=None, op0=OP.mult, op1=OP.bypass,
                        )
                        nc.vector.tensor_tensor(
                            out=l_t, in0=z_t, in1=u_t, op=OP.max,
                        )
                    else:
                        nc.scalar.activation(
                            out=l_t, in_=tq_t, func=AF.Prelu,
                            bias=s2ptr, scale=1.0, alpha=NEG_SLOPE,
                        )
                # one exp over the whole group: amortizes the ACT access
                # overhead on the saturated engine
                nc.scalar.activation(out=pt2, in_=l2, func=AF.Exp)
                if len(mm_pending) >= mm_delay:
                    emit_group_matmuls(*mm_pending.pop(0))
                mm_pending.append((CH0[b] + c0, pt2))

            # ---- deadline-sorted emission: each producer unit is emitted
            # when the E cursor (in global chunks) reaches its due chunk, so
            # every in-order engine queue sees stages in data-arrival order ----
            units = []
            xdue = {}
            for q in range(n // (XCH * 128)):
                # front-loaded: all x lands in the first ~16 E chunks, where
                # compute is DMA-bound and idle. Cadence 4 chunks so a parked
                # x DMA (xbt slot reuse) never starves adj on the SP queue.
                due = -100 if q == 0 else 4 * q - la_x
                xdue[q] = due
                units.append((due, 0, "x", q))
            for b, cpj in enumerate(BLKS):
                nd = max(1, cpj // 2)
                for d in range(nd):
                    # last granule of block b lands la_adj chunks before the
                    # block's E groups start (E needs the full column block)
                    units.append(
                        (CH0[b] - la_adj + 2 * (d + 1 - nd), 1, "adj", (b, d))
                    )
            units.append((-99.5, 2, "s12", None))
            for a in range(NCH // 2):
                # front-loaded like x: A-slices chew through the early
                # DMA-bound idle so the drain has no A work left
                due = -99 + a if a < SUB // 2 else max(
                    2 * a + 1 - la_a, xdue[a // (XCH // 2)] + 0.5
                )
                units.append((due, 2, "A", a))
            for b, cpj in enumerate(BLKS):
                nd = max(1, cpj // 2)
                R = SUB // nd
                for s in range(SUB):
                    # 1-chunk spacing; the LAST cast of block b is emitted
                    # la_cast chunks before the block's first E group (which
                    # needs all 8 casts: each chunk's transposes touch every
                    # zm row-tile). Never before its own adj granule.
                    adj_due = CH0[b] - la_adj + 2 * (s // R + 1 - nd)
                    units.append(
                        (
                            max(CH0[b] - la_cast - (SUB - 1 - s), adj_due + 0.5),
                            3,
                            "cast",
                            (b, s),
                        )
                    )
            units.sort(key=lambda u: (u[0], u[1]))

            ui = 0

            def drain_units(e):
                nonlocal ui
                while ui < len(units) and units[ui][0] <= e:
                    _, _, kind, payload = units[ui]
                    ui += 1
                    if kind == "x":
                        emit_xdma(payload)
                    elif kind == "adj":
                        emit_adj(*payload)
                    elif kind == "A":
                        emit_A_slice(payload)
                    elif kind == "s12":
                        emit_s12()
                    else:
                        emit_cast(*payload)

            def chunk_to_bc(g):
                for b in range(len(BLKS)):
                    if CH0[b] <= g < CH0[b + 1]:
                        return b, g - CH0[b]
                return None

            e = 0
            tcur = 0    # transpose cursor (global chunks)
            for b, cpj in enumerate(BLKS):
                for c0 in range(0, cpj, EG):
                    drain_units(e)
                    while tcur < min(e + EG + tq_lead, NCH):
                        emit_T(*chunk_to_bc(tcur))
                        tcur += 1
                    emit_PX(b, c0)
                    e += EG
            drain_units(10**9)
            while mm_pending:
                emit_group_matmuls(*mm_pending.pop(0))

        # ---- phase C: normalize + elu in h'^T space (big [128, I_BLK] ops,
        # rowsum broadcast across partitions by a rank-1 ones matmul), then
        # transpose + store ----
        with ExitStack() as cctx:
            fpool = cctx.enter_context(tc.tile_pool(name="fpool", bufs=2))
            fps = cctx.enter_context(tc.tile_pool(name="fps", bufs=2, space="PSUM"))
            NSUB = I_BLK // 128
            for ih in range(NIH):
                rinv1 = fpool.tile([1, I_BLK], F32, tag="rinv1")
                nc.vector.reciprocal(rinv1, rs_ps[ih])
                rinv16 = fpool.tile([1, I_BLK], p_dt, tag="rinv16")
                nc.vector.tensor_copy(rinv16, rinv1)
                rinv_ps = fps.tile([128, I_BLK], F32, tag="bc")
                nc.tensor.matmul(
                    rinv_ps, lhsT=ones128[:1, :], rhs=rinv16,
                    start=True, stop=True,
                )
                # t1/t2 read acc from PSUM, so the broadcast reciprocal must
                # come from SBUF (one PSUM operand per instruction)
                rinv = fpool.tile([128, I_BLK], F32, tag="rinv")
                nc.vector.tensor_copy(rinv, rinv_ps)
                # elu(v), v = acc/rowsum: relu(v) + exp(min(v, 0)) - 1,
                # with relu(v) = (acc max 0) * rinv and min(v,0) = (acc min 0) * rinv
                t1 = fpool.tile([128, I_BLK], F32, tag="t1")
                nc.vector.scalar_tensor_tensor(
                    out=t1, in0=acc_ps[ih], scalar=0.0, in1=rinv,
                    op0=OP.max, op1=OP.mult,
                )
                t2 = fpool.tile([128, I_BLK], F32, tag="t2")
                nc.vector.scalar_tensor_tensor(
                    out=t2, in0=acc_ps[ih], scalar=0.0, in1=rinv,
                    op0=OP.min, op1=OP.mult,
                )
                t3 = fpool.tile([128, I_BLK], F32, tag="t3")
                nc.scalar.activation(out=t3, in_=t2, func=AF.Exp)
                o_t = fpool.tile([128, I_BLK], F32, tag="o")
                nc.vector.scalar_tensor_tensor(
                    out=o_t, in0=t3, scalar=-1.0, in1=t1, op0=OP.add, op1=OP.add
                )
                tp = fps.tile([128, I_BLK], F32, tag="fps")
                for s in range(NSUB):
                    nc.tensor.transpose(
                        tp[:, s * 128 : (s + 1) * 128],
                        o_t[:, s * 128 : (s + 1) * 128],
                        ident32,
                    )
                o_sb = fpool.tile([128, I_BLK], F32, tag="osb")
                nc.vector.tensor_copy(o_sb, tp)
                nc.scalar.dma_start(
                    dram3(
                        out_ap, ih * I_BLK * f_out,
                        [[f_out, 128], [128 * f_out, NSUB], [1, f_out]],
                    ),
                    o_sb.rearrange("p (s f) -> p s f", s=NSUB),
                )

    nc.compile()
    return nc


_CACHE = {}


def _compiled_full():
    if "nc" not in _CACHE:
        _CACHE["nc"] = build_gat()
    return _CACHE["nc"]


def make_in_maps(x, W, a, adj):
    rows = N_FULL // N_CORES
    in_maps = []
    for c in range(N_CORES):
        sl = slice(c * rows, (c + 1) * rows)
        in_maps.append(
            {
                "x": np.ascontiguousarray(np.roll(x, -c * rows, axis=0)),
                "w": W,
                "a": a,
                "adj": np.ascontiguousarray(np.roll(adj[sl], -c * rows, axis=1)),
            }
        )
    return in_maps


def kernel(x, W, a, adj):
    from concourse.bass_utils import run_bass_kernel_spmd

    nc = _compiled_full()
    x = np.ascontiguousarray(np.asarray(x, dtype=np.float32))
    W = np.ascontiguousarray(np.asarray(W, dtype=np.float32))
    a = np.ascontiguousarray(np.asarray(a, dtype=np.float32))
    adj = np.asarray(adj)
    assert adj.dtype == np.int32
    in_maps = make_in_maps(x, W, a, adj)
    res = run_bass_kernel_spmd(nc, in_maps, core_ids=list(range(N_CORES)))
    out = np.concatenate([res.results[c]["out"] for c in range(N_CORES)], axis=0)
    return out.astype(np.float32)


# revision 63
# speedup vs baseline: 1.0256x; 1.0061x over previous
"""GATv2 layer kernel for Trainium2 — 8 NeuronCores, SPMD row-sharded.

Math (reference):
    h = x @ W
    s1 = h @ a[:F];  s2 = h @ a[F:]
    e  = leaky_relu(s1[:,None] + s2[None,:], 0.2)
    e  = where(adj > 0, e, -9e15)
    att = softmax(e, axis=1)
    out = elu(att @ h)

Kernel strategy (per core, rows of adj/out sharded across 8 cores; x and
adj columns are rotated per core on the host so each core's own rows are
always chunks 0..SUB-1 — one SPMD program, no separate xs input):
  - s1/s2 are separable: s1 = x @ (W @ a1), s2 = x @ (W @ a2); each core
    computes full h (fp16) from the replicated (rotated) x.
  - exponents are tiny (|s1+s2| <~ 5) so softmax needs no max-subtraction:
    P = adj * exp(lrelu(z)) realized as exp(lrelu(z + adjL)) with
    adjL = (adj-1)*60000 (exp of ~-1.2e4 underflows to exactly 0).
  - the int32->fp16 cast of adj folds BOTH the mask affine and the s1 bias
    in one gpsimd pass: zm = adj*BIG + (s1 - BIG)  (per-partition ptr bias).
  - zm tiles are PE-transposed to [j, i] layout BEFORE the nonlinearity, so
    lrelu(+s2)/exp run in transposed orientation and exp writes the
    attention-matmul rhs (P^T) straight to SBUF — no PSUM->SBUF copy pass.
  - softmax row sums come from a ones-vector matmul accumulated in PSUM
    alongside the attention matmul (both PSUM rows share one bank).
  - j blocks have VARIABLE width ([2,2,4] chunks then 8-chunk blocks): the
    first exp group only needs a 2-chunk column block, so the ACT engine
    starts ~8us earlier than with uniform 8-chunk blocks.
  - all input DMAs are issued from the SP queue in exact consumption order
    (x block 0, adj block 0, ...); emission is deadline-sorted so every
    engine's in-order queue sees pipeline stages in data-arrival order.
  - final: normalize + elu in h'^T orientation as a few big [128, 512] ops
    (rowsum reciprocal is broadcast across partitions with a rank-1 ones
    matmul), then transpose and stream to DRAM.
"""

import sys

if "/opt/trn_rl_repo" not in sys.path:
    sys.path.insert(0, "/opt/trn_rl_repo")

from contextlib import ExitStack

import numpy as np

import concourse.bass as bass
import concourse.tile as tile
from concourse import bacc, mybir
from concourse.masks import make_identity

F32 = mybir.dt.float32
F16 = mybir.dt.float16
I32 = mybir.dt.int32
AF = mybir.ActivationFunctionType
OP = mybir.AluOpType

N_FULL = 8192
F_IN = 256
F_OUT = 128
N_CORES = 8
NEG_SLOPE = 0.2
MASK_BIG = 60000.0  # exactly representable in fp16; exp(-0.2*60000) == 0


def build_gat(
    n=N_FULL,
    rows=N_FULL // N_CORES,
    f_in=F_IN,
    f_out=F_OUT,
    blks=(2, 2, 4, 8, 8, 8, 8, 8, 8, 8),   # chunks per j block
    dve_every=2,     # every k-th chunk routes lrelu to DVE instead of ACT
    cast_split="ppdpppdp",  # cast engine by row-tile index (p=Pool, a=ACT, d=DVE)
    zm_ring={"zmp": 12, "zma": 0, "zmd": 4, "zms": 4},
    p_dt=F16,
    adj_bufs=6,
    zm_bufs=20,
    pt_bufs=7,
    ep_bufs=5,
    tq_bufs=3,
    pa_bufs=2,
    x_bufs=5,
    xt_bufs=4,
    eg=2,
    la_x=12,         # emission lookaheads, in global 128-col chunks
    la_adj=10,
    la_a=18,
    la_cast=6,
    mm_delay=3,      # groups of slack between exp and its attention matmuls
    tq_lead=1,       # extra chunks the tq transposes run ahead of prelu/exp
    drain_from=52,   # chunk index where drain routing kicks in
    drain_mod=3,     # in drain, route (mod-1)/mod of lrelu groups to DVE
):
    """Build the per-core Bass program. All cores run the identical program;
    per-core behavior comes only from per-core input data (adj shard + the
    host-side rotation of x / adj columns). Returns the compiled module."""
    KC = f_in // 128          # k chunks of f_in
    NCH = n // 128            # column chunks of adj / row chunks of h
    SUB = rows // 128         # i subtiles per core
    I_BLK = min(512, rows)
    NIH = rows // I_BLK       # i halves for matmul psum banks
    FO2 = f_out + 2           # h columns + [s1 s2]
    EG = eg                   # chunks per exp/matmul-delay group
    XCH = 8                   # chunks per x DMA block
    BLKS = list(blks)
    assert sum(BLKS) == NCH and all(c % EG == 0 for c in BLKS)
    CH0 = [0]
    for c in BLKS:
        CH0.append(CH0[-1] + c)

    nc = bacc.Bacc(
        "TRN2",
        target_bir_lowering=False,
        debug=False,
        enable_asserts=False,
        num_devices=1,
    )
    x_ap = nc.dram_tensor("x", [n, f_in], F32, kind="ExternalInput").ap()
    w_ap = nc.dram_tensor("w", [f_in, f_out], F32, kind="ExternalInput").ap()
    a_ap = nc.dram_tensor("a", [2 * f_out, 1], F32, kind="ExternalInput").ap()
    adj_ap = nc.dram_tensor("adj", [rows, n], I32, kind="ExternalInput").ap()
    out_ap = nc.dram_tensor("out", [rows, f_out], F32, kind="ExternalOutput").ap()

    def dram3(ap, off, dims):
        return bass.AP(tensor=ap.tensor, offset=ap.offset + off, ap=dims)

    with tile.TileContext(nc) as tc, ExitStack() as ctx:
        singles = ctx.enter_context(tc.tile_pool(name="singles", bufs=1))

        rhsW = singles.tile([128, KC * FO2], F32)   # per kc: [W chunk | w1 w2]
        ident32 = singles.tile([128, 128], F32)
        make_identity(nc, ident32)
        identp = singles.tile([128, 128], p_dt)
        make_identity(nc, identp)
        h_sb = singles.tile([128, NCH * f_out], p_dt)
        s2st = singles.tile([128, NCH], F32)     # s2[j] in [j%128, j//128]
        s1m = singles.tile([128, SUB], F32)      # s1 - BIG (cast bias ptr)
        ones128 = singles.tile([128, 128], p_dt)
        scratch = singles.tile([128, f_out], F32)
        a1b = singles.tile([128, f_out], F32)
        a2b = singles.tile([128, f_out], F32)

        # ---- constants: rhsW leads the SP DMA queue (everything chains off
        # rhsW16 -> h -> s1m -> casts); a1b/a2b arrive in parallel on ACT ----
        nc.gpsimd.memset(ones128, 1.0)
        nc.scalar.dma_start(a1b, dram3(a_ap, 0, [[0, 128], [1, f_out]]))
        nc.scalar.dma_start(a2b, dram3(a_ap, f_out, [[0, 128], [1, f_out]]))
        for kc in range(KC):
            nc.sync.dma_start(
                rhsW[:, kc * FO2 : kc * FO2 + f_out],
                w_ap[kc * 128 : (kc + 1) * 128, :],
            )
        # w1 = W @ a1, w2 = W @ a2 appended as columns of rhsW
        # (NOTE tensor_tensor_reduce crashes the device — use scalar_tensor_tensor)
        for kc in range(KC):
            for ai, ab in ((0, a1b), (1, a2b)):
                nc.vector.scalar_tensor_tensor(
                    out=scratch,
                    in0=rhsW[:, kc * FO2 : kc * FO2 + f_out],
                    scalar=1.0,
                    in1=ab,
                    op0=OP.mult,
                    op1=OP.mult,
                    accum_out=rhsW[:, kc * FO2 + f_out + ai : kc * FO2 + f_out + ai + 1],
                )
        rhsW16 = singles.tile([128, KC * FO2], p_dt)
        # w1/w2 broadcast across partitions ([128, k] each) lets s1 (cast
        # bias) and the first chunks' s2 be computed straight from the
        # arriving x tile with multiply+accumulate ops — skipping the
        # transpose->matmul chain that otherwise gates the whole ramp
        w12bc = singles.tile([128, 2 * f_in], F32)
        wrow16 = singles.tile([1, 2 * f_in], p_dt)
        s1raw = singles.tile([128, SUB], F32)
        jd = singles.tile([128, f_in], F32)
        jp = singles.tile([128, f_in], F32)

        with tc.tile_pool(name="wprep", bufs=1, space="PSUM") as wpp:
            wb = wpp.tile([1, 2 * f_in], F32, tag="wb")
            for ai in range(2):
                for kc in range(KC):
                    nc.tensor.transpose(
                        wb[:, ai * f_in + kc * 128 : ai * f_in + (kc + 1) * 128],
                        rhsW[:, kc * FO2 + f_out + ai : kc * FO2 + f_out + ai + 1],
                        ident32,
                    )
            nc.vector.tensor_copy(wrow16, wb)
            wbc = wpp.tile([128, 2 * f_in], F32, tag="wbc")
            for ai in range(2):
                nc.tensor.matmul(
                    wbc[:, ai * f_in : (ai + 1) * f_in],
                    lhsT=ones128[:1, :],
                    rhs=wrow16[:, ai * f_in : (ai + 1) * f_in],
                    start=True,
                    stop=True,
                )
            nc.vector.tensor_copy(w12bc, wbc)

        acc_pool = ctx.enter_context(tc.tile_pool(name="acc", bufs=1, space="PSUM"))
        acc_ps = [
            acc_pool.tile([128, I_BLK], F32, name=f"acc{ih}", tag=f"acc{ih}")
            for ih in range(NIH)
        ]
        # both rowsum accumulators share one PSUM bank at partition
        # offsets 0 and 64 (legal matmul tile positions for M=1)
        rs_bank = acc_pool.tile([128, I_BLK], F32, name="rs_bank", tag="rs_bank")
        rs_ps = [rs_bank[64 * ih : 64 * ih + 1, :] for ih in range(NIH)]

        with ExitStack() as bctx:
            xpool = bctx.enter_context(tc.tile_pool(name="xpool", bufs=x_bufs))
            xtp = bctx.enter_context(tc.tile_pool(name="xtp", bufs=xt_bufs))
            pa_ps = bctx.enter_context(tc.tile_pool(name="pa_ps", bufs=pa_bufs, space="PSUM"))
            tqp = bctx.enter_context(tc.tile_pool(name="tqp", bufs=tq_bufs, space="PSUM"))
            adjp = bctx.enter_context(tc.tile_pool(name="adjp", bufs=adj_bufs))
            zmp = bctx.enter_context(tc.tile_pool(name="zmp", bufs=zm_bufs))
            ptp = bctx.enter_context(tc.tile_pool(name="ptp", bufs=pt_bufs))
            ep = bctx.enter_context(tc.tile_pool(name="ep", bufs=ep_bufs))

            nc.vector.tensor_copy(rhsW16, rhsW)

            xq_tiles = {}

            def emit_xdma(q):
                xbt = xpool.tile([128, XCH * f_in], F32, tag="xbt")
                nc.sync.dma_start(
                    xbt,
                    dram3(
                        x_ap,
                        q * XCH * 128 * f_in,
                        [[f_in, 128], [128 * f_in, XCH], [1, f_in]],
                    ),
                )
                xq_tiles[q] = xbt

            def emit_A_slice(a):
                """Two x chunks (2a, 2a+1): fp32 transposes, one fp16 staging
                copy, h matmuls, h/s2 (and s1-BIG for own rows) stashes."""
                xbt = xq_tiles[a // (XCH // 2)]
                pr = a % (XCH // 2)
                ic0 = 2 * a
                own = ic0 < SUB
                tp = pa_ps.tile([128, 2 * f_in], F32, tag="pa")
                for cc in range(2):
                    c = 2 * pr + cc
                    for kc in range(KC):
                        nc.tensor.transpose(
                            tp[:, cc * f_in + kc * 128 : cc * f_in + kc * 128 + 128],
                            xbt[:, c * f_in + kc * 128 : c * f_in + (kc + 1) * 128],
                            ident32,
                        )
                xT2 = xtp.tile([128, 2 * f_in], p_dt, tag="xT")
                nc.vector.tensor_copy(xT2, tp)
                hps_full = pa_ps.tile([128, 2 * f_in], F32, tag="pa", name=f"hps_{a}")
                hps = hps_full[:, : 2 * FO2]
                for cc in range(2):
                    for kc in range(KC):
                        nc.tensor.matmul(
                            hps[:, cc * FO2 : (cc + 1) * FO2],
                            lhsT=xT2[:, cc * f_in + kc * 128 : cc * f_in + (kc + 1) * 128],
                            rhs=rhsW16[:, kc * FO2 : (kc + 1) * FO2],
                            start=(kc == 0),
                            stop=(kc == KC - 1),
                        )
                h2 = hps.rearrange("p (c f) -> p c f", c=2)
                nc.vector.tensor_copy(
                    h_sb[:, ic0 * f_out : (ic0 + 2) * f_out].rearrange(
                        "p (c f) -> p c f", c=2
                    ),
                    h2[:, :, :f_out],
                )
                if not own:
                    # own chunks' s2 (and s1) come from the ramp-time
                    # multiply+accumulate path instead. The copy lives on ACT:
                    # it precedes its consumer exps in ACT's own in-order
                    # queue, so it can never gate them from another engine.
                    nc.vector.tensor_copy(
                        s2st[:, ic0 : ic0 + 2].rearrange("p (c f) -> p c f", c=2),
                        h2[:, :, f_out + 1 : f_out + 2],
                    )

            def emit_s12():
                """s1 (cast bias) and own-chunk s2 directly from x block 0:
                accum_out of x*w_bc sums over k. DVE takes s1 (it gates every
                cast), Pool takes s2 (only chunk c's prelu needs col c)."""
                xbt = xq_tiles[0]
                for c in range(SUB):
                    xs = xbt[:, c * f_in : (c + 1) * f_in]
                    nc.vector.scalar_tensor_tensor(
                        out=jd, in0=xs, scalar=1.0, in1=w12bc[:, :f_in],
                        op0=OP.mult, op1=OP.mult,
                        accum_out=s1raw[:, c : c + 1],
                    )
                    nc.vector.scalar_tensor_tensor(
                        out=jp, in0=xs, scalar=1.0, in1=w12bc[:, f_in:],
                        op0=OP.mult, op1=OP.mult,
                        accum_out=s2st[:, c : c + 1],
                    )
                    if c % 4 == 3:
                        nc.vector.tensor_scalar(
                            out=s1m[:, c - 3 : c + 1], in0=s1raw[:, c - 3 : c + 1],
                            scalar1=-MASK_BIG, scalar2=None,
                            op0=OP.add, op1=OP.bypass,
                        )

            # adj DMA granules: always 1MB ([128, R, jb] with R*jb == 2048),
            # viewed at the block's chunk width
            adj_views = {}

            def emit_adj(b, d):
                cpj = BLKS[b]
                jb = 128 * cpj
                nd = max(1, cpj // 2)
                R = 8 // nd
                t = adjp.tile([128, 2048], I32, tag="adj", name=f"adj_{b}_{d}")
                v = t.rearrange("p (r j) -> p r j", r=R)
                nc.sync.dma_start(
                    v,
                    dram3(
                        adj_ap,
                        CH0[b] * 128 + d * R * 128 * n,
                        [[n, 128], [128 * n, R], [1, jb]],
                    ),
                )
                adj_views.setdefault(b, {})[d] = v

            # zm tiles pack ceil(1024/jb) s-subtiles per [128, 1024] buffer.
            # The engine routing is a FIXED function of the tile index so each
            # engine recycles its own zm slot ring (tag per engine) — slot
            # WAW reuse then never couples one engine's queue to another's.
            zm_tiles = {}

            def emit_cast(b, s):
                cpj = BLKS[b]
                jb = 128 * cpj
                spt = max(1, 1024 // jb)      # s-subtiles packed per zm tile
                nd = max(1, cpj // 2)
                R = 8 // nd
                tl = zm_tiles.setdefault(b, {})
                ti = s // spt
                if spt == 1:
                    eng = cast_split[ti % len(cast_split)]
                    tag = f"zm{eng}"
                else:
                    # startup small blocks: dedicated ring, engines spread
                    eng = "ppadppad"[s % 8]
                    tag = "zms"
                if ti not in tl:
                    tl[ti] = zmp.tile(
                        [128, 1024], p_dt, tag=tag, name=f"zm_{b}_{ti}",
                        bufs=zm_ring[tag],
                    )
                zm = tl[ti][:, (s % spt) * jb : (s % spt + 1) * jb]
                asl = adj_views[b][s // R][:, s % R, :]
                if eng == "a":
                    # Prelu with alpha=1 == identity affine with ptr bias
                    nc.scalar.activation(
                        out=zm, in_=asl, func=AF.Prelu,
                        bias=s1m[:, s : s + 1], scale=MASK_BIG, alpha=1.0,
                    )
                elif eng == "d":
                    nc.vector.tensor_scalar(
                        out=zm, in0=asl, scalar1=MASK_BIG,
                        scalar2=s1m[:, s : s + 1], op0=OP.mult, op1=OP.add,
                    )
                else:
                    nc.gpsimd.tensor_scalar(
                        out=zm, in0=asl, scalar1=MASK_BIG,
                        scalar2=s1m[:, s : s + 1], op0=OP.mult, op1=OP.add,
                    )

            ucount = [0]
            mm_pending = []

            def emit_group_matmuls(g0, pt2):
                pt3 = pt2.rearrange("p (t n) -> p t n", t=EG)
                for ih in range(NIH):
                    rsl = pt3[:, :, ih * I_BLK : (ih + 1) * I_BLK]
                    for t in range(EG):
                        nc.tensor.matmul(
                            acc_ps[ih],
                            lhsT=h_sb[:, (g0 + t) * f_out : (g0 + t + 1) * f_out],
                            rhs=rsl[:, t, :],
                            start=(g0 == 0 and t == 0),
                            stop=(g0 == NCH - EG and t == EG - 1),
                            skip_group_check=True,
                        )
                        nc.tensor.matmul(
                            rs_ps[ih],
                            lhsT=ones128[:, :1],
                            rhs=rsl[:, t, :],
                            start=(g0 == 0 and t == 0),
                            stop=(g0 == NCH - EG and t == EG - 1),
                            skip_group_check=True,
                        )

            tq_by_chunk = {}

            def emit_T(b, c):
                """PE transposes of one chunk into a tq PSUM tile; runs
                tq_lead chunks ahead of the prelu/exp consumers so ACT never
                waits on PE at block boundaries."""
                cpj = BLKS[b]
                jb = 128 * cpj
                spt = max(1, 1024 // jb)
                tq_t = tqp.tile([128, rows], p_dt, tag="tq", name=f"tq_{b}_{c}")
                for s in range(SUB):
                    nc.tensor.transpose(
                        tq_t[:, s * 128 : (s + 1) * 128],
                        zm_tiles[b][s // spt][
                            :, (s % spt) * jb + c * 128 : (s % spt) * jb + (c + 1) * 128
                        ],
                        identp,
                    )
                tq_by_chunk[CH0[b] + c] = tq_t

            def emit_PX(b, c0):
                """exp(lrelu(z)) == max(e^z, (e^z)^0.2) since exp is
                monotone: ONE ACT op (Exp with the s2 ptr bias, read straight
                from PSUM tq), then a fast-mode pow and a tensor max on DVE.
                Then the (delayed) matmuls of an earlier group."""
                pt2 = ptp.tile([128, EG * rows], p_dt, tag="pt")
                l2 = ep.tile([128, EG * rows], p_dt, tag="l", bufs=3)
                # in the drain (A copies done) DVE has slack while ACT
                # saturates: route lrelu to DVE more aggressively there
                k = ucount[0]
                if CH0[b] + c0 >= drain_from:
                    use_dve = k % drain_mod != drain_mod - 1
                else:
                    use_dve = dve_every > 0 and (k % dve_every == 0)
                ucount[0] += 1
                for c in range(c0, c0 + EG):
                    g = CH0[b] + c
                    tq_t = tq_by_chunk.pop(g)
                    s2ptr = s2st[:, g : g + 1]
                    l_t = l2[:, (c - c0) * rows : (c - c0 + 1) * rows]
                    if use_dve:
                        # lrelu via fast-mode ops only: z+s2 (2x, PSUM in),
                        # 0.2*z (4x), tensor max (2x) — the stt form has no
                        # DVE fast mode and costs ~2x more
                        z_t = ep.tile([128, rows], p_dt, tag="z", bufs=2)
                        nc.vector.tensor_scalar(
                            out=z_t, in0=tq_t, scalar1=s2ptr,
                            scalar2=None, op0=OP.add, op1=OP.bypass,
                        )
                        u_t = ep.tile([128, rows], p_dt, tag="u", bufs=2)
                        nc.vector.tensor_scalar(
                            out=u_t, in0=z_t, scalar1=NEG_SLOPE,
                            scalar2=None, op0=OP.mult, op1=OP.bypass,
                        )
                        nc.vector.tensor_tensor(
                            out=l_t, in0=z_t, in1=u_t, op=OP.max,
                        )
                    else:
                        nc.scalar.activation(
                            out=l_t, in_=tq_t, func=AF.Prelu,
                            bias=s2ptr, scale=1.0, alpha=NEG_SLOPE,
                        )
                # one exp over the whole group: amortizes the ACT access
                # overhead on the saturated engine
                nc.scalar.activation(out=pt2, in_=l2, func=AF.Exp)
                if len(mm_pending) >= mm_delay:
                    emit_group_matmuls(*mm_pending.pop(0))
                mm_pending.append((CH0[b] + c0, pt2))

            # ---- deadline-sorted emission: each producer unit is emitted
            # when the E cursor (in global chunks) reaches its due chunk, so
            # every in-order engine queue sees stages in data-arrival order ----
            units = []
            xdue = {}
            for q in range(n // (XCH * 128)):
                # front-loaded: all x lands in the first ~16 E chunks, where
                # compute is DMA-bound and idle. Cadence 4 chunks so a parked
                # x DMA (xbt slot reuse) never starves adj on the SP queue.
                due = -100 if q == 0 else 4 * q - la_x
                xdue[q] = due
                units.append((due, 0, "x", q))
            for b, cpj in enumerate(BLKS):
                nd = max(1, cpj // 2)
                for d in range(nd):
                    # last granule of block b lands la_adj chunks before the
                    # block's E groups start (E needs the full column block)
                    units.append(
                        (CH0[b] - la_adj + 2 * (d + 1 - nd), 1, "adj", (b, d))
                    )
            units.append((-99.5, 2, "s12", None))
            for a in range(NCH // 2):
                # front-loaded like x: A-slices chew through the early
                # DMA-bound idle so the drain has no A work left
                due = -99 + a if a < SUB // 2 else max(
                    2 * a + 1 - la_a, xdue[a // (XCH // 2)] + 0.5
                )
                units.append((due, 2, "A", a))
            for b, cpj in enumerate(BLKS):
                nd = max(1, cpj // 2)
                R = SUB // nd
                for s in range(SUB):
                    # 1-chunk spacing; the LAST cast of block b is emitted
                    # la_cast chunks before the block's first E group (which
                    # needs all 8 casts: each chunk's transposes touch every
                    # zm row-tile). Never before its own adj granule.
                    adj_due = CH0[b] - la_adj + 2 * (s // R + 1 - nd)
                    units.append(
                        (
                            max(CH0[b] - la_cast - (SUB - 1 - s), adj_due + 0.5),
                            3,
                            "cast",
                            (b, s),
                        )
                    )
            units.sort(key=lambda u: (u[0], u[1]))

            ui = 0

            def drain_units(e):
                nonlocal ui
                while ui < len(units) and units[ui][0] <= e:
                    _, _, kind, payload = units[ui]
                    ui += 1
                    if kind == "x":
                        emit_xdma(payload)
                    elif kind == "adj":
                        emit_adj(*payload)
                    elif kind == "A":
                        emit_A_slice(payload)
                    elif kind == "s12":
                        emit_s12()
                    else:
                        emit_cast(*payload)

            def chunk_to_bc(g):
                for b in range(len(BLKS)):
                    if CH0[b] <= g < CH0[b + 1]:
                        return b, g - CH0[b]
                return None

            e = 0
            tcur = 0    # transpose cursor (global chunks)
            for b, cpj in enumerate(BLKS):
                for c0 in range(0, cpj, EG):
                    drain_units(e)
                    while tcur < min(e + EG + tq_lead, NCH):
                        emit_T(*chunk_to_bc(tcur))
                        tcur += 1
                    emit_PX(b, c0)
                    e += EG
            drain_units(10**9)
            while mm_pending:
                emit_group_matmuls(*mm_pending.pop(0))

        # ---- phase C: normalize + elu in h'^T space (big [128, I_BLK] ops,
        # rowsum broadcast across partitions by a rank-1 ones matmul), then
        # transpose + store ----
        with ExitStack() as cctx:
            fpool = cctx.enter_context(tc.tile_pool(name="fpool", bufs=2))
            fps = cctx.enter_context(tc.tile_pool(name="fps", bufs=2, space="PSUM"))
            NSUB = I_BLK // 128
            for ih in range(NIH):
                rinv1 = fpool.tile([1, I_BLK], F32, tag="rinv1")
                nc.vector.reciprocal(rinv1, rs_ps[ih])
                rinv16 = fpool.tile([1, I_BLK], p_dt, tag="rinv16")
                nc.vector.tensor_copy(rinv16, rinv1)
                rinv_ps = fps.tile([128, I_BLK], F32, tag="bc")
                nc.tensor.matmul(
                    rinv_ps, lhsT=ones128[:1, :], rhs=rinv16,
                    start=True, stop=True,
                )
                # t1/t2 read acc from PSUM, so the broadcast reciprocal must
                # come from SBUF (one PSUM operand per instruction)
                rinv = fpool.tile([128, I_BLK], F32, tag="rinv")
                nc.vector.tensor_copy(rinv, rinv_ps)
                # elu(v), v = acc/rowsum: relu(v) + exp(min(v, 0)) - 1,
                # with relu(v) = (acc max 0) * rinv and min(v,0) = (acc min 0) * rinv
                t1 = fpool.tile([128, I_BLK], F32, tag="t1")
                nc.vector.scalar_tensor_tensor(
                    out=t1, in0=acc_ps[ih], scalar=0.0, in1=rinv,
                    op0=OP.max, op1=OP.mult,
                )
                t2 = fpool.tile([128, I_BLK], F32, tag="t2")
                nc.vector.scalar_tensor_tensor(
                    out=t2, in0=acc_ps[ih], scalar=0.0, in1=rinv,
                    op0=OP.min, op1=OP.mult,
                )
                t3 = fpool.tile([128, I_BLK], F32, tag="t3")
                nc.scalar.activation(out=t3, in_=t2, func=AF.Exp)
                o_t = fpool.tile([128, I_BLK], F32, tag="o")
                nc.vector.scalar_tensor_tensor(
                    out=o_t, in0=t3, scalar=-1.0, in1=t1, op0=OP.add, op1=OP.add
                )
                tp = fps.tile([128, I_BLK], F32, tag="fps")
                for s in range(NSUB):
                    nc.tensor.transpose(
                        tp[:, s * 128 : (s + 1) * 128],
                        o_t[:, s * 128 : (s + 1) * 128],
                        ident32,
                    )
                o_sb = fpool.tile([128, I_BLK], F32, tag="osb")
                nc.vector.tensor_copy(o_sb, tp)
                nc.scalar.dma_start(
                    dram3(
                        out_ap, ih * I_BLK * f_out,
                        [[f_out, 128], [128 * f_out, NSUB], [1, f_out]],
                    ),
                    o_sb.rearrange("p (s f) -> p s f", s=NSUB),
                )

    nc.compile()
    return nc


_CACHE = {}


def _compiled_full():
    if "nc" not in _CACHE:
        _CACHE["nc"] = build_gat()
    return _CACHE["nc"]


def make_in_maps(x, W, a, adj):
    rows = N_FULL // N_CORES
    in_maps = []
    for c in range(N_CORES):
        sl = slice(c * rows, (c + 1) * rows)
        in_maps.append(
            {
                "x": np.ascontiguousarray(np.roll(x, -c * rows, axis=0)),
                "w": W,
                "a": a,
                "adj": np.ascontiguousarray(np.roll(adj[sl], -c * rows, axis=1)),
            }
        )
    return in_maps


def kernel(x, W, a, adj):
    from concourse.bass_utils import run_bass_kernel_spmd

    nc = _compiled_full()
    x = np.ascontiguousarray(np.asarray(x, dtype=np.float32))
    W = np.ascontiguousarray(np.asarray(W, dtype=np.float32))
    a = np.ascontiguousarray(np.asarray(a, dtype=np.float32))
    adj = np.asarray(adj)
    assert adj.dtype == np.int32
    in_maps = make_in_maps(x, W, a, adj)
    res = run_bass_kernel_spmd(nc, in_maps, core_ids=list(range(N_CORES)))
    out = np.concatenate([res.results[c]["out"] for c in range(N_CORES)], axis=0)
    return out.astype(np.float32)


# revision 65
# speedup vs baseline: 1.0301x; 1.0044x over previous
"""GATv2 layer kernel for Trainium2 — 8 NeuronCores, SPMD row-sharded.

Math (reference):
    h = x @ W
    s1 = h @ a[:F];  s2 = h @ a[F:]
    e  = leaky_relu(s1[:,None] + s2[None,:], 0.2)
    e  = where(adj > 0, e, -9e15)
    att = softmax(e, axis=1)
    out = elu(att @ h)

Kernel strategy (per core, rows of adj/out sharded across 8 cores; x and
adj columns are rotated per core on the host so each core's own rows are
always chunks 0..SUB-1 — one SPMD program, no separate xs input):
  - s1/s2 are separable: s1 = x @ (W @ a1), s2 = x @ (W @ a2); each core
    computes full h (fp16) from the replicated (rotated) x.
  - exponents are tiny (|s1+s2| <~ 5) so softmax needs no max-subtraction:
    P = adj * exp(lrelu(z)) realized as exp(lrelu(z + adjL)) with
    adjL = (adj-1)*60000 (exp of ~-1.2e4 underflows to exactly 0).
  - the int32->fp16 cast of adj folds BOTH the mask affine and the s1 bias
    in one gpsimd pass: zm = adj*BIG + (s1 - BIG)  (per-partition ptr bias).
  - zm tiles are PE-transposed to [j, i] layout BEFORE the nonlinearity, so
    lrelu(+s2)/exp run in transposed orientation and exp writes the
    attention-matmul rhs (P^T) straight to SBUF — no PSUM->SBUF copy pass.
  - softmax row sums come from a ones-vector matmul accumulated in PSUM
    alongside the attention matmul (both PSUM rows share one bank).
  - j blocks have VARIABLE width ([2,2,4] chunks then 8-chunk blocks): the
    first exp group only needs a 2-chunk column block, so the ACT engine
    starts ~8us earlier than with uniform 8-chunk blocks.
  - all input DMAs are issued from the SP queue in exact consumption order
    (x block 0, adj block 0, ...); emission is deadline-sorted so every
    engine's in-order queue sees pipeline stages in data-arrival order.
  - final: normalize + elu in h'^T orientation as a few big [128, 512] ops
    (rowsum reciprocal is broadcast across partitions with a rank-1 ones
    matmul), then transpose and stream to DRAM.
"""

import sys

if "/opt/trn_rl_repo" not in sys.path:
    sys.path.insert(0, "/opt/trn_rl_repo")

from contextlib import ExitStack

import numpy as np

import concourse.bass as bass
import concourse.tile as tile
from concourse import bacc, mybir
from concourse.masks import make_identity

F32 = mybir.dt.float32
F16 = mybir.dt.float16
I32 = mybir.dt.int32
AF = mybir.ActivationFunctionType
OP = mybir.AluOpType

N_FULL = 8192
F_IN = 256
F_OUT = 128
N_CORES = 8
NEG_SLOPE = 0.2
MASK_BIG = 60000.0  # exactly representable in fp16; exp(-0.2*60000) == 0


def build_gat(
    n=N_FULL,
    rows=N_FULL // N_CORES,
    f_in=F_IN,
    f_out=F_OUT,
    blks=(2, 2, 4, 8, 8, 8, 8, 8, 8, 8),   # chunks per j block
    dve_every=2,     # every k-th chunk routes lrelu to DVE instead of ACT
    cast_split="ppdpppdp",  # cast engine by row-tile index (p=Pool, a=ACT, d=DVE)
    zm_ring={"zmp": 12, "zma": 0, "zmd": 4, "zms": 4},
    p_dt=F16,
    adj_bufs=6,
    zm_bufs=20,
    pt_bufs=7,
    ep_bufs=5,
    tq_bufs=3,
    pa_bufs=2,
    x_bufs=5,
    xt_bufs=4,
    eg=2,
    la_x=12,         # emission lookaheads, in global 128-col chunks
    la_adj=10,
    la_a=18,
    la_cast=6,
    mm_delay=3,      # groups of slack between exp and its attention matmuls
    tq_lead=1,       # extra chunks the tq transposes run ahead of prelu/exp
    drain_from=52,   # chunk index where drain routing kicks in
    drain_mod=3,     # in drain, route (mod-1)/mod of lrelu groups to DVE
    front_until=0,   # before this chunk, only 1/3 of lrelu groups go to DVE
    cast_split_early="papappap",
    cast_early_until=0,
):
    """Build the per-core Bass program. All cores run the identical program;
    per-core behavior comes only from per-core input data (adj shard + the
    host-side rotation of x / adj columns). Returns the compiled module."""
    KC = f_in // 128          # k chunks of f_in
    NCH = n // 128            # column chunks of adj / row chunks of h
    SUB = rows // 128         # i subtiles per core
    I_BLK = min(512, rows)
    NIH = rows // I_BLK       # i halves for matmul psum banks
    FO2 = f_out + 2           # h columns + [s1 s2]
    EG = eg                   # chunks per exp/matmul-delay group
    XCH = 8                   # chunks per x DMA block
    BLKS = list(blks)
    assert sum(BLKS) == NCH and all(c % EG == 0 for c in BLKS)
    CH0 = [0]
    for c in BLKS:
        CH0.append(CH0[-1] + c)

    nc = bacc.Bacc(
        "TRN2",
        target_bir_lowering=False,
        debug=False,
        enable_asserts=False,
        num_devices=1,
    )
    x_ap = nc.dram_tensor("x", [n, f_in], F32, kind="ExternalInput").ap()
    w_ap = nc.dram_tensor("w", [f_in, f_out], F32, kind="ExternalInput").ap()
    a_ap = nc.dram_tensor("a", [2 * f_out, 1], F32, kind="ExternalInput").ap()
    adj_ap = nc.dram_tensor("adj", [rows, n], I32, kind="ExternalInput").ap()
    out_ap = nc.dram_tensor("out", [rows, f_out], F32, kind="ExternalOutput").ap()

    def dram3(ap, off, dims):
        return bass.AP(tensor=ap.tensor, offset=ap.offset + off, ap=dims)

    with tile.TileContext(nc) as tc, ExitStack() as ctx:
        singles = ctx.enter_context(tc.tile_pool(name="singles", bufs=1))

        rhsW = singles.tile([128, KC * FO2], F32)   # per kc: [W chunk | w1 w2]
        ident32 = singles.tile([128, 128], F32)
        make_identity(nc, ident32)
        identp = singles.tile([128, 128], p_dt)
        make_identity(nc, identp)
        h_sb = singles.tile([128, NCH * f_out], p_dt)
        s2st = singles.tile([128, NCH], F32)     # s2[j] in [j%128, j//128]
        s1m = singles.tile([128, SUB], F32)      # s1 - BIG (cast bias ptr)
        ones128 = singles.tile([128, 128], p_dt)
        scratch = singles.tile([128, f_out], F32)
        ab12 = singles.tile([128, 2 * f_out], F32)
        a1b = ab12[:, :f_out]
        a2b = ab12[:, f_out:]

        # ---- constants: rhsW leads the SP DMA queue (everything chains off
        # rhsW16 -> h -> s1m -> casts); a1b/a2b arrive in parallel on ACT ----
        nc.gpsimd.memset(ones128, 1.0)
        nc.scalar.dma_start(
            ab12, dram3(a_ap, 0, [[0, 128], [f_out, 2], [1, f_out]])
        )
        nc.sync.dma_start(
            rhsW.rearrange("p (kc f) -> p kc f", kc=KC)[:, :, :f_out],
            dram3(w_ap, 0, [[f_out, 128], [128 * f_out, KC], [1, f_out]]),
        )
        # w1 = W @ a1, w2 = W @ a2 appended as columns of rhsW
        # (NOTE tensor_tensor_reduce crashes the device — use scalar_tensor_tensor)
        for kc in range(KC):
            for ai, ab in ((0, a1b), (1, a2b)):
                nc.vector.scalar_tensor_tensor(
                    out=scratch,
                    in0=rhsW[:, kc * FO2 : kc * FO2 + f_out],
                    scalar=1.0,
                    in1=ab,
                    op0=OP.mult,
                    op1=OP.mult,
                    accum_out=rhsW[:, kc * FO2 + f_out + ai : kc * FO2 + f_out + ai + 1],
                )
        rhsW16 = singles.tile([128, KC * FO2], p_dt)
        # w1/w2 broadcast across partitions ([128, k] each) lets s1 (cast
        # bias) and the first chunks' s2 be computed straight from the
        # arriving x tile with multiply+accumulate ops — skipping the
        # transpose->matmul chain that otherwise gates the whole ramp
        w12bc = singles.tile([128, 2 * f_in], F32)
        wrow16 = singles.tile([1, 2 * f_in], p_dt)
        s1raw = singles.tile([128, SUB], F32)
        jd = singles.tile([128, f_in], F32)
        jp = singles.tile([128, f_in], F32)

        with tc.tile_pool(name="wprep", bufs=1, space="PSUM") as wpp:
            wb = wpp.tile([1, 2 * f_in], F32, tag="wb")
            for ai in range(2):
                for kc in range(KC):
                    nc.tensor.transpose(
                        wb[:, ai * f_in + kc * 128 : ai * f_in + (kc + 1) * 128],
                        rhsW[:, kc * FO2 + f_out + ai : kc * FO2 + f_out + ai + 1],
                        ident32,
                    )
            nc.vector.tensor_copy(wrow16, wb)
            wbc = wpp.tile([128, 2 * f_in], F32, tag="wbc")
            for ai in range(2):
                nc.tensor.matmul(
                    wbc[:, ai * f_in : (ai + 1) * f_in],
                    lhsT=ones128[:1, :],
                    rhs=wrow16[:, ai * f_in : (ai + 1) * f_in],
                    start=True,
                    stop=True,
                )
            nc.vector.tensor_copy(w12bc, wbc)

        acc_pool = ctx.enter_context(tc.tile_pool(name="acc", bufs=1, space="PSUM"))
        acc_ps = [
            acc_pool.tile([128, I_BLK], F32, name=f"acc{ih}", tag=f"acc{ih}")
            for ih in range(NIH)
        ]
        # both rowsum accumulators share one PSUM bank at partition
        # offsets 0 and 64 (legal matmul tile positions for M=1)
        rs_bank = acc_pool.tile([128, I_BLK], F32, name="rs_bank", tag="rs_bank")
        rs_ps = [rs_bank[64 * ih : 64 * ih + 1, :] for ih in range(NIH)]

        with ExitStack() as bctx:
            xpool = bctx.enter_context(tc.tile_pool(name="xpool", bufs=x_bufs))
            xtp = bctx.enter_context(tc.tile_pool(name="xtp", bufs=xt_bufs))
            pa_ps = bctx.enter_context(tc.tile_pool(name="pa_ps", bufs=pa_bufs, space="PSUM"))
            tqp = bctx.enter_context(tc.tile_pool(name="tqp", bufs=tq_bufs, space="PSUM"))
            adjp = bctx.enter_context(tc.tile_pool(name="adjp", bufs=adj_bufs))
            zmp = bctx.enter_context(tc.tile_pool(name="zmp", bufs=zm_bufs))
            ptp = bctx.enter_context(tc.tile_pool(name="ptp", bufs=pt_bufs))
            ep = bctx.enter_context(tc.tile_pool(name="ep", bufs=ep_bufs))

            nc.vector.tensor_copy(rhsW16, rhsW)

            xq_tiles = {}

            def emit_xdma(q):
                xbt = xpool.tile([128, XCH * f_in], F32, tag="xbt")
                nc.sync.dma_start(
                    xbt,
                    dram3(
                        x_ap,
                        q * XCH * 128 * f_in,
                        [[f_in, 128], [128 * f_in, XCH], [1, f_in]],
                    ),
                )
                xq_tiles[q] = xbt

            def emit_A_slice(a):
                """Two x chunks (2a, 2a+1): fp32 transposes, one fp16 staging
                copy, h matmuls, h/s2 (and s1-BIG for own rows) stashes."""
                xbt = xq_tiles[a // (XCH // 2)]
                pr = a % (XCH // 2)
                ic0 = 2 * a
                own = ic0 < SUB
                tp = pa_ps.tile([128, 2 * f_in], F32, tag="pa")
                for cc in range(2):
                    c = 2 * pr + cc
                    for kc in range(KC):
                        nc.tensor.transpose(
                            tp[:, cc * f_in + kc * 128 : cc * f_in + kc * 128 + 128],
                            xbt[:, c * f_in + kc * 128 : c * f_in + (kc + 1) * 128],
                            ident32,
                        )
                xT2 = xtp.tile([128, 2 * f_in], p_dt, tag="xT")
                nc.vector.tensor_copy(xT2, tp)
                hps_full = pa_ps.tile([128, 2 * f_in], F32, tag="pa", name=f"hps_{a}")
                hps = hps_full[:, : 2 * FO2]
                for cc in range(2):
                    for kc in range(KC):
                        nc.tensor.matmul(
                            hps[:, cc * FO2 : (cc + 1) * FO2],
                            lhsT=xT2[:, cc * f_in + kc * 128 : cc * f_in + (kc + 1) * 128],
                            rhs=rhsW16[:, kc * FO2 : (kc + 1) * FO2],
                            start=(kc == 0),
                            stop=(kc == KC - 1),
                        )
                h2 = hps.rearrange("p (c f) -> p c f", c=2)
                nc.vector.tensor_copy(
                    h_sb[:, ic0 * f_out : (ic0 + 2) * f_out].rearrange(
                        "p (c f) -> p c f", c=2
                    ),
                    h2[:, :, :f_out],
                )
                if not own:
                    # own chunks' s2 (and s1) come from the ramp-time
                    # multiply+accumulate path instead. The copy lives on ACT:
                    # it precedes its consumer exps in ACT's own in-order
                    # queue, so it can never gate them from another engine.
                    nc.vector.tensor_copy(
                        s2st[:, ic0 : ic0 + 2].rearrange("p (c f) -> p c f", c=2),
                        h2[:, :, f_out + 1 : f_out + 2],
                    )

            def emit_s12():
                """s1 (cast bias) and own-chunk s2 directly from x block 0:
                accum_out of x*w_bc sums over k. DVE takes s1 (it gates every
                cast), Pool takes s2 (only chunk c's prelu needs col c)."""
                xbt = xq_tiles[0]
                for c in range(SUB):
                    xs = xbt[:, c * f_in : (c + 1) * f_in]
                    nc.vector.scalar_tensor_tensor(
                        out=jd, in0=xs, scalar=1.0, in1=w12bc[:, :f_in],
                        op0=OP.mult, op1=OP.mult,
                        accum_out=s1raw[:, c : c + 1],
                    )
                    nc.vector.scalar_tensor_tensor(
                        out=jp, in0=xs, scalar=1.0, in1=w12bc[:, f_in:],
                        op0=OP.mult, op1=OP.mult,
                        accum_out=s2st[:, c : c + 1],
                    )
                    if c % 4 == 3:
                        nc.vector.tensor_scalar(
                            out=s1m[:, c - 3 : c + 1], in0=s1raw[:, c - 3 : c + 1],
                            scalar1=-MASK_BIG, scalar2=None,
                            op0=OP.add, op1=OP.bypass,
                        )

            # adj DMA granules: always 1MB ([128, R, jb] with R*jb == 2048),
            # viewed at the block's chunk width
            adj_views = {}

            def emit_adj(b, d):
                cpj = BLKS[b]
                jb = 128 * cpj
                nd = max(1, cpj // 2)
                R = 8 // nd
                t = adjp.tile([128, 2048], I32, tag="adj", name=f"adj_{b}_{d}")
                v = t.rearrange("p (r j) -> p r j", r=R)
                nc.sync.dma_start(
                    v,
                    dram3(
                        adj_ap,
                        CH0[b] * 128 + d * R * 128 * n,
                        [[n, 128], [128 * n, R], [1, jb]],
                    ),
                )
                adj_views.setdefault(b, {})[d] = v

            # zm tiles pack ceil(1024/jb) s-subtiles per [128, 1024] buffer.
            # The engine routing is a FIXED function of the tile index so each
            # engine recycles its own zm slot ring (tag per engine) — slot
            # WAW reuse then never couples one engine's queue to another's.
            zm_tiles = {}

            def emit_cast(b, s):
                cpj = BLKS[b]
                jb = 128 * cpj
                spt = max(1, 1024 // jb)      # s-subtiles packed per zm tile
                nd = max(1, cpj // 2)
                R = 8 // nd
                tl = zm_tiles.setdefault(b, {})
                ti = s // spt
                if spt == 1:
                    # front blocks lean on ACT (starved there), drain blocks
                    # avoid it (saturated there)
                    split = cast_split_early if CH0[b] < cast_early_until else cast_split
                    eng = split[ti % len(split)]
                    tag = f"zm{eng}"
                else:
                    # startup small blocks: dedicated ring, engines spread
                    eng = "ppadppad"[s % 8]
                    tag = "zms"
                if ti not in tl:
                    tl[ti] = zmp.tile(
                        [128, 1024], p_dt, tag=tag, name=f"zm_{b}_{ti}",
                        bufs=zm_ring[tag],
                    )
                zm = tl[ti][:, (s % spt) * jb : (s % spt + 1) * jb]
                asl = adj_views[b][s // R][:, s % R, :]
                if eng == "a":
                    # Prelu with alpha=1 == identity affine with ptr bias
                    nc.scalar.activation(
                        out=zm, in_=asl, func=AF.Prelu,
                        bias=s1m[:, s : s + 1], scale=MASK_BIG, alpha=1.0,
                    )
                elif eng == "d":
                    nc.vector.tensor_scalar(
                        out=zm, in0=asl, scalar1=MASK_BIG,
                        scalar2=s1m[:, s : s + 1], op0=OP.mult, op1=OP.add,
                    )
                else:
                    nc.gpsimd.tensor_scalar(
                        out=zm, in0=asl, scalar1=MASK_BIG,
                        scalar2=s1m[:, s : s + 1], op0=OP.mult, op1=OP.add,
                    )

            ucount = [0]
            mm_pending = []

            def emit_group_matmuls(g0, pt2):
                pt3 = pt2.rearrange("p (t n) -> p t n", t=EG)
                for ih in range(NIH):
                    rsl = pt3[:, :, ih * I_BLK : (ih + 1) * I_BLK]
                    for t in range(EG):
                        nc.tensor.matmul(
                            acc_ps[ih],
                            lhsT=h_sb[:, (g0 + t) * f_out : (g0 + t + 1) * f_out],
                            rhs=rsl[:, t, :],
                            start=(g0 == 0 and t == 0),
                            stop=(g0 == NCH - EG and t == EG - 1),
                            skip_group_check=True,
                        )
                        nc.tensor.matmul(
                            rs_ps[ih],
                            lhsT=ones128[:, :1],
                            rhs=rsl[:, t, :],
                            start=(g0 == 0 and t == 0),
                            stop=(g0 == NCH - EG and t == EG - 1),
                            skip_group_check=True,
                        )

            tq_by_chunk = {}

            def emit_T(b, c):
                """PE transposes of one chunk into a tq PSUM tile; runs
                tq_lead chunks ahead of the prelu/exp consumers so ACT never
                waits on PE at block boundaries."""
                cpj = BLKS[b]
                jb = 128 * cpj
                spt = max(1, 1024 // jb)
                tq_t = tqp.tile([128, rows], p_dt, tag="tq", name=f"tq_{b}_{c}")
                for s in range(SUB):
                    nc.tensor.transpose(
                        tq_t[:, s * 128 : (s + 1) * 128],
                        zm_tiles[b][s // spt][
                            :, (s % spt) * jb + c * 128 : (s % spt) * jb + (c + 1) * 128
                        ],
                        identp,
                    )
                tq_by_chunk[CH0[b] + c] = tq_t

            def emit_PX(b, c0):
                """exp(lrelu(z)) == max(e^z, (e^z)^0.2) since exp is
                monotone: ONE ACT op (Exp with the s2 ptr bias, read straight
                from PSUM tq), then a fast-mode pow and a tensor max on DVE.
                Then the (delayed) matmuls of an earlier group."""
                pt2 = ptp.tile([128, EG * rows], p_dt, tag="pt")
                l2 = ep.tile([128, EG * rows], p_dt, tag="l", bufs=3)
                # in the drain (A copies done) DVE has slack while ACT
                # saturates: route lrelu to DVE more aggressively there
                k = ucount[0]
                g0 = CH0[b] + c0
                if g0 >= drain_from:
                    use_dve = k % drain_mod != drain_mod - 1
                elif g0 < front_until:
                    use_dve = k % 3 == 0
                else:
                    use_dve = dve_every > 0 and (k % dve_every == 0)
                ucount[0] += 1
                for c in range(c0, c0 + EG):
                    g = CH0[b] + c
                    tq_t = tq_by_chunk.pop(g)
                    s2ptr = s2st[:, g : g + 1]
                    l_t = l2[:, (c - c0) * rows : (c - c0 + 1) * rows]
                    if use_dve:
                        # lrelu via fast-mode ops only: z+s2 (2x, PSUM in),
                        # 0.2*z (4x), tensor max (2x) — the stt form has no
                        # DVE fast mode and costs ~2x more
                        z_t = ep.tile([128, rows], p_dt, tag="z", bufs=2)
                        nc.vector.tensor_scalar(
                            out=z_t, in0=tq_t, scalar1=s2ptr,
                            scalar2=None, op0=OP.add, op1=OP.bypass,
                        )
                        u_t = ep.tile([128, rows], p_dt, tag="u", bufs=2)
                        nc.vector.tensor_scalar(
                            out=u_t, in0=z_t, scalar1=NEG_SLOPE,
                            scalar2=None, op0=OP.mult, op1=OP.bypass,
                        )
                        nc.vector.tensor_tensor(
                            out=l_t, in0=z_t, in1=u_t, op=OP.max,
                        )
                    else:
                        nc.scalar.activation(
                            out=l_t, in_=tq_t, func=AF.Prelu,
                            bias=s2ptr, scale=1.0, alpha=NEG_SLOPE,
                        )
                # one exp over the whole group: amortizes the ACT access
                # overhead on the saturated engine
                nc.scalar.activation(out=pt2, in_=l2, func=AF.Exp)
                if len(mm_pending) >= mm_delay:
                    emit_group_matmuls(*mm_pending.pop(0))
                mm_pending.append((CH0[b] + c0, pt2))

            # ---- deadline-sorted emission: each producer unit is emitted
            # when the E cursor (in global chunks) reaches its due chunk, so
            # every in-order engine queue sees stages in data-arrival order ----
            units = []
            xdue = {}
            for q in range(n // (XCH * 128)):
                # front-loaded: all x lands in the first ~16 E chunks, where
                # compute is DMA-bound and idle. Cadence 4 chunks so a parked
                # x DMA (xbt slot reuse) never starves adj on the SP queue.
                due = -100 if q == 0 else 4 * q - la_x
                xdue[q] = due
                units.append((due, 0, "x", q))
            for b, cpj in enumerate(BLKS):
                nd = max(1, cpj // 2)
                for d in range(nd):
                    # last granule of block b lands la_adj chunks before the
                    # block's E groups start (E needs the full column block)
                    units.append(
                        (CH0[b] - la_adj + 2 * (d + 1 - nd), 1, "adj", (b, d))
                    )
            units.append((-99.5, 2, "s12", None))
            for a in range(NCH // 2):
                # front-loaded like x: A-slices chew through the early
                # DMA-bound idle so the drain has no A work left
                due = -99 + a if a < SUB // 2 else max(
                    2 * a + 1 - la_a, xdue[a // (XCH // 2)] + 0.5
                )
                units.append((due, 2, "A", a))
            for b, cpj in enumerate(BLKS):
                nd = max(1, cpj // 2)
                R = SUB // nd
                for s in range(SUB):
                    # 1-chunk spacing; the LAST cast of block b is emitted
                    # la_cast chunks before the block's first E group (which
                    # needs all 8 casts: each chunk's transposes touch every
                    # zm row-tile). Never before its own adj granule.
                    adj_due = CH0[b] - la_adj + 2 * (s // R + 1 - nd)
                    units.append(
                        (
                            max(CH0[b] - la_cast - (SUB - 1 - s), adj_due + 0.5),
                            3,
                            "cast",
                            (b, s),
                        )
                    )
            units.sort(key=lambda u: (u[0], u[1]))

            ui = 0

            def drain_units(e):
                nonlocal ui
                while ui < len(units) and units[ui][0] <= e:
                    _, _, kind, payload = units[ui]
                    ui += 1
                    if kind == "x":
                        emit_xdma(payload)
                    elif kind == "adj":
                        emit_adj(*payload)
                    elif kind == "A":
                        emit_A_slice(payload)
                    elif kind == "s12":
                        emit_s12()
                    else:
                        emit_cast(*payload)

            def chunk_to_bc(g):
                for b in range(len(BLKS)):
                    if CH0[b] <= g < CH0[b + 1]:
                        return b, g - CH0[b]
                return None

            e = 0
            tcur = 0    # transpose cursor (global chunks)
            for b, cpj in enumerate(BLKS):
                for c0 in range(0, cpj, EG):
                    drain_units(e)
                    while tcur < min(e + EG + tq_lead, NCH):
                        emit_T(*chunk_to_bc(tcur))
                        tcur += 1
                    emit_PX(b, c0)
                    e += EG
            drain_units(10**9)
            while mm_pending:
                emit_group_matmuls(*mm_pending.pop(0))

        # ---- phase C: normalize + elu in h'^T space (big [128, I_BLK] ops,
        # rowsum broadcast across partitions by a rank-1 ones matmul), then
        # transpose + store ----
        with ExitStack() as cctx:
            fpool = cctx.enter_context(tc.tile_pool(name="fpool", bufs=2))
            fps = cctx.enter_context(tc.tile_pool(name="fps", bufs=2, space="PSUM"))
            NSUB = I_BLK // 128
            for ih in range(NIH):
                rinv1 = fpool.tile([1, I_BLK], F32, tag="rinv1")
                nc.vector.reciprocal(rinv1, rs_ps[ih])
                rinv16 = fpool.tile([1, I_BLK], p_dt, tag="rinv16")
                nc.vector.tensor_copy(rinv16, rinv1)
                rinv_ps = fps.tile([128, I_BLK], F32, tag="bc")
                nc.tensor.matmul(
                    rinv_ps, lhsT=ones128[:1, :], rhs=rinv16,
                    start=True, stop=True,
                )
                # t1/t2 read acc from PSUM, so the broadcast reciprocal must
                # come from SBUF (one PSUM operand per instruction)
                rinv = fpool.tile([128, I_BLK], F32, tag="rinv")
                nc.vector.tensor_copy(rinv, rinv_ps)
                # elu(v), v = acc/rowsum: relu(v) + exp(min(v, 0)) - 1,
                # with relu(v) = (acc max 0) * rinv and min(v,0) = (acc min 0) * rinv
                t1 = fpool.tile([128, I_BLK], F32, tag="t1")
                nc.vector.scalar_tensor_tensor(
                    out=t1, in0=acc_ps[ih], scalar=0.0, in1=rinv,
                    op0=OP.max, op1=OP.mult,
                )
                t2 = fpool.tile([128, I_BLK], F32, tag="t2")
                nc.vector.scalar_tensor_tensor(
                    out=t2, in0=acc_ps[ih], scalar=0.0, in1=rinv,
                    op0=OP.min, op1=OP.mult,
                )
                t3 = fpool.tile([128, I_BLK], F32, tag="t3")
                nc.scalar.activation(out=t3, in_=t2, func=AF.Exp)
                o_t = fpool.tile([128, I_BLK], F32, tag="o")
                nc.vector.scalar_tensor_tensor(
                    out=o_t, in0=t3, scalar=-1.0, in1=t1, op0=OP.add, op1=OP.add
                )
                tp = fps.tile([128, I_BLK], F32, tag="fps")
                for s in range(NSUB):
                    nc.tensor.transpose(
                        tp[:, s * 128 : (s + 1) * 128],
                        o_t[:, s * 128 : (s + 1) * 128],
                        ident32,
                    )
                o_sb = fpool.tile([128, I_BLK], F32, tag="osb")
                nc.vector.tensor_copy(o_sb, tp)
                nc.scalar.dma_start(
                    dram3(
                        out_ap, ih * I_BLK * f_out,
                        [[f_out, 128], [128 * f_out, NSUB], [1, f_out]],
                    ),
                    o_sb.rearrange("p (s f) -> p s f", s=NSUB),
                )

    nc.compile()
    return nc


_CACHE = {}


def _compiled_full():
    if "nc" not in _CACHE:
        _CACHE["nc"] = build_gat()
    return _CACHE["nc"]


def make_in_maps(x, W, a, adj):
    rows = N_FULL // N_CORES
    in_maps = []
    for c in range(N_CORES):
        sl = slice(c * rows, (c + 1) * rows)
        in_maps.append(
            {
                "x": np.ascontiguousarray(np.roll(x, -c * rows, axis=0)),
                "w": W,
                "a": a,
                "adj": np.ascontiguousarray(np.roll(adj[sl], -c * rows, axis=1)),
            }
        )
    return in_maps


def kernel(x, W, a, adj):
    from concourse.bass_utils import run_bass_kernel_spmd

    nc = _compiled_full()
    x = np.ascontiguousarray(np.asarray(x, dtype=np.float32))
    W = np.ascontiguousarray(np.asarray(W, dtype=np.float32))
    a = np.ascontiguousarray(np.asarray(a, dtype=np.float32))
    adj = np.asarray(adj)
    assert adj.dtype == np.int32
    in_maps = make_in_maps(x, W, a, adj)
    res = run_bass_kernel_spmd(nc, in_maps, core_ids=list(range(N_CORES)))
    out = np.concatenate([res.results[c]["out"] for c in range(N_CORES)], axis=0)
    return out.astype(np.float32)
